# revision 1
# baseline (speedup 1.0000x reference)
"""Fused multi-head attention block (QKV proj + softmax attention + out-proj
+ LayerNorm) for Trainium2, sharded over 8 NeuronCores.

Sharding: tensor-parallel over heads. Core c owns heads [4c, 4c+4).
  - Each core computes q/k/v for its heads over the whole sequence
    (both batches), attention in S^T (keys-on-partitions) layout.
  - The kernel is scheduled as a continuous Scalar-engine exp stream
    (the hard bottleneck: 33.5M exps/core at 128 lanes x 1.2 GHz) with a
    software-pipelined S -> exp -> PV/denominator loop per key-tile, so
    the PE never sleeps long enough to lose its HAM warm clock.
  - Attention output is normalized (1/den) on the producer core, then an
    AllToAll (split into 4 chunks, overlapped with compute) reshards
    from head-parallel to row-parallel (4 x 128 rows per core); each core
    applies the 1024x1024 output projection + bias + LayerNorm.

dtypes: fp32 storage; QKV and S^T matmuls run as float32r (TF32-like,
full PE rate at N>=256); exp(S^T), PV and out-proj use bf16.
LayerNorm rstd uses exp(-0.5*ln(var+eps)) so the Scalar engine keeps a
single activation table set (natural_log_exp_and_others) loaded.
"""
import sys

for _p in ("/opt/trn_rl_repo", "/root/.axon_site/_ro/trn_rl_repo"):
    if _p not in sys.path:
        sys.path.insert(0, _p)

import numpy as np

import concourse.bass as bass
import concourse.tile as tile
from concourse import bacc, mybir
from concourse.masks import make_identity

F32 = mybir.dt.float32
F32R = mybir.dt.float32r
BF16 = mybir.dt.bfloat16
AF = mybir.ActivationFunctionType
ALU = mybir.AluOpType

N_CORES = 8
B, N, DIM = 2, 2048, 1024
HEADS, DH = 32, 32           # 32 heads x 32 dim/head
HPC = HEADS // N_CORES       # 4 heads per core
ROWS = B * N                 # 4096 global rows
SCALE = DH ** -0.5
EPS = 1e-6
KT = N // 128                # 16 key tiles per batch
QB = 512                     # q-block width
NQB = N // QB                # 4 q-blocks per batch
RC = 512                     # projection row-chunk
NRC = ROWS // RC             # 16 row chunks
NCK = 4                      # a2a chunks; chunk k = global rows [1024k, 1024k+1024)
                             # core c owns rows 1024k + 128c .. +128 of each chunk


def _build(debug=False):
    nc = bacc.Bacc("TRN2", target_bir_lowering=False, debug=False,
                   num_devices=N_CORES)

    xT_d = nc.dram_tensor("xT", [DIM, ROWS], F32R, kind="ExternalInput").ap()
    wqkv_d = nc.dram_tensor("wqkv", [DIM, 3 * HPC * DH], F32R,
                            kind="ExternalInput").ap()
    wout_d = nc.dram_tensor("wout", [DIM, DIM], F32, kind="ExternalInput").ap()
    bout_d = nc.dram_tensor("bout", [DIM], F32, kind="ExternalInput").ap()
    gamma_d = nc.dram_tensor("gamma", [DIM], F32, kind="ExternalInput").ap()
    beta_d = nc.dram_tensor("beta", [DIM], F32, kind="ExternalInput").ap()
    # rows: [chunk(4), 128]; global row = 1024*ck + 128*core + r
    out_d = nc.dram_tensor("out", [NCK * 128, DIM], F32,
                           kind="ExternalOutput").ap()
    if debug:
        dbg_qT = nc.dram_tensor("dbg_qT", [128, ROWS], F32,
                                kind="ExternalOutput").ap()
        dbg_kT = nc.dram_tensor("dbg_kT", [128, ROWS], F32,
                                kind="ExternalOutput").ap()
        dbg_V = nc.dram_tensor("dbg_V", [128, 2 * KT, 128], BF16,
                               kind="ExternalOutput").ap()
        dbg_att = nc.dram_tensor("dbg_att", [2 * NQB, 128, QB], BF16,
                                 kind="ExternalOutput").ap()
        dbg_rec = nc.dram_tensor("dbg_rec", [2 * NQB, 128, QB], F32,
                                 kind="ExternalOutput").ap()
        dbg_a2o = nc.dram_tensor("dbg_a2o", [NCK, N_CORES, 128, 128], BF16,
                                 kind="ExternalOutput").ap()

    with tile.TileContext(nc) as tc:
        with (
            tc.tile_pool(name="const", bufs=1) as const,
            tc.tile_pool(name="work", bufs=1) as work,
            tc.tile_pool(name="ps", bufs=1, space="PSUM") as ps,
            tc.tile_pool(name="dram", bufs=1, space="DRAM") as dram,
        ):
            # ---------------- constants / weights ----------------
            wqkv_sb = const.tile([128, 8, 3 * HPC * DH], F32R)
            nc.sync.dma_start(
                wqkv_sb[:], wqkv_d.rearrange("(kc p) m -> p kc m", p=128))
            ones_bf = const.tile([128, 1], BF16)
            nc.vector.memset(ones_bf[:], 1.0)
            ident = const.tile([128, 128], F32)
            make_identity(nc, ident[:])
            eps_sb = const.tile([128, 1], F32)
            nc.vector.memset(eps_sb[:], EPS)
            # head->partition-group broadcast matrix:
            # eb2[k, m] = 1 iff k == 32*(m//32); contracting against a
            # [128, q] tile whose rows 32h hold per-head denominators
            # broadcasts row 32h to output partitions [32h, 32h+32).
            eb2 = const.tile([128, 128], F32)
            nc.vector.memset(eb2[:], 0.0)
            for h in range(4):
                nc.vector.memset(eb2[32 * h:32 * h + 1, 32 * h:32 * h + 32],
                                 1.0)
            rec_full = const.tile([128, QB], F32)
            nc.vector.memset(rec_full[:], 0.0)
            dn_st = const.tile([128, QB], F32)
            nc.vector.memset(dn_st[:], 1.0)
            # warm the ACT table set (ln first so the shared
            # natural_log_exp_and_others set is chosen, then exp)
            scr = const.tile([128, 1], F32)
            nc.scalar.activation(out=scr[:], in_=eps_sb[:], func=AF.Ln,
                                 bias=eps_sb[:], scale=1.0)
            nc.scalar.activation(out=scr[:], in_=eps_sb[:], func=AF.Exp,
                                 scale=1.0)
            # row-broadcast vectors [128, 1024]; DMAs deferred past the
            # startup window so they don't queue ahead of the xT loads
            bout_bc = const.tile([128, DIM], F32)
            gamma_bc = const.tile([128, DIM], F32)
            beta_bc = const.tile([128, DIM], F32)

            def load_ln_consts():
                for bc, src_d in ((bout_bc, bout_d), (gamma_bc, gamma_d),
                                  (beta_bc, beta_d)):
                    nc.gpsimd.dma_start(out=bc[:], in_=bass.AP(
                        tensor=src_d.tensor, offset=src_d.offset,
                        ap=[[0, 128], [1, DIM]]))
            # w_out -> bf16 [128, 8, 1024]
            wout_bf = const.tile([128, 8, DIM], BF16)

            # ---------------- persistent activations ----------------
            qT_sb = const.tile([128, ROWS], F32R)   # 4h x 32d on partitions
            kT_sb = const.tile([128, ROWS], F32R)
            V_sb = const.tile([128, 2 * KT, 128], BF16)  # [key%128, ktile, ch]

            # ---------------- dram bounce buffers ----------------
            a2a_in = [dram.tile([N_CORES, 128, 128], BF16, name=f"a2ai_{k}")
                      for k in range(NCK)]
            a2a_out = [dram.tile([N_CORES, 128, 128], BF16, name=f"a2ao_{k}")
                       for k in range(NCK)]

            # ---------------- phase A: projections ----------------
            def proj_rowchunk(rc):
                xt = work.tile([128, 8, RC], F32R, tag="xt", bufs=3,
                               name=f"xt_{rc}")
                dma_eng = nc.sync if rc % 2 == 0 else nc.gpsimd
                dma_eng.dma_start(
                    xt[:],
                    xT_d[:, rc * RC:(rc + 1) * RC]
                    .rearrange("(kc p) n -> p kc n", p=128))
                for name, mofs, dst in (("q", 0, qT_sb), ("k", 128, kT_sb)):
                    pp = ps.tile([128, RC], F32, tag="sp", bufs=3,
                                 name=f"pp_{name}_{rc}")
                    for kc in range(8):
                        nc.tensor.matmul(
                            pp[:], wqkv_sb[:, kc, mofs:mofs + 128],
                            xt[:, kc, :], start=(kc == 0), stop=(kc == 7))
                    nc.vector.tensor_copy(dst[:, rc * RC:(rc + 1) * RC], pp[:])
                # v: project (vT layout), cast bf16, DMA-transpose into V_sb
                pv_ = ps.tile([128, RC], F32, tag="sp", bufs=3,
                               name=f"pp_v_{rc}")
                for kc in range(8):
                    nc.tensor.matmul(
                        pv_[:], wqkv_sb[:, kc, 256:384], xt[:, kc, :],
                        start=(kc == 0), stop=(kc == 7))
                # one buffer per row-chunk: the async DMA-transpose read of
                # vt is not WAR-tracked, so never reuse these buffers
                vt = work.tile([128, RC], F32, tag="vt", bufs=2,
                               name=f"vt_{rc}")
                nc.vector.tensor_copy(vt[:], pv_[:])
                for i in range(RC // 128):
                    tp = ps.tile([128, 128], F32, tag="sp", bufs=3,
                                 name=f"tp_{rc}_{i}")
                    nc.tensor.matmul(
                        tp[:], vt[:, i * 128:(i + 1) * 128], ident[:],
                        is_transpose=True, start=True, stop=True)
                    nc.vector.tensor_copy(
                        V_sb[:, rc * (RC // 128) + i, :], tp[:])

            def load_wout(j):
                st = work.tile([128, DIM], F32, tag="wstage", bufs=2,
                               name=f"wst_{j}")
                nc.sync.dma_start(st[:], wout_d[j * 128:(j + 1) * 128, :])
                nc.vector.tensor_copy(wout_bf[:, j, :], st[:])

            proj_rowchunk(0)                # enough rows to start qb0

            # ---------------- phase B: attention ----------------
            # Software-pipelined per q-block: S two key-tiles ahead, exp
            # paces the loop, PV + denominator trail by one tile.
            def emit_S(b, qb, kt):
                q0 = b * N + qb * QB
                k0 = b * N + kt * 128
                tA = ps.tile([128, 2, QB], F32, tag="sp", bufs=3,
                             name=f"sA_{b}_{qb}_{kt}")
                tB = ps.tile([128, 2, QB], F32, tag="sp", bufs=3,
                             name=f"sB_{b}_{qb}_{kt}")
                for h in range(4):
                    t = tA if h < 2 else tB
                    nc.tensor.matmul(
                        t[:, h % 2, :],
                        kT_sb[32 * h:32 * h + 32, k0:k0 + 128],
                        qT_sb[32 * h:32 * h + 32, q0:q0 + QB],
                        start=True, stop=True, tile_position=(32 * h, 0))
                return tA, tB

            pending_epi = [None]

            def flush_epi():
                if pending_epi[0] is not None:
                    fn = pending_epi[0]
                    pending_epi[0] = None
                    fn()

            def attention_qblock(b, qb, extra=None):
                pvp = ps.tile([128, QB], F32, tag="pv", name=f"pv_{b}_{qb}")
                dnp = ps.tile([128, QB], F32, tag="dn", name=f"dn_{b}_{qb}")
                s_tiles = {0: emit_S(b, qb, 0), 1: emit_S(b, qb, 1)}
                flush_epi()   # previous q-block's tail, behind our first S
                for kt in range(KT):
                    if extra is not None:
                        extra(kt)
                    if kt + 2 < KT:
                        s_tiles[kt + 2] = emit_S(b, qb, kt + 2)
                    tA, tB = s_tiles.pop(kt)
                    eA = work.tile([128, 2, QB], BF16, tag="expt", bufs=8,
                                   name=f"eA_{b}_{qb}_{kt}")
                    eB = work.tile([128, 2, QB], BF16, tag="expt", bufs=8,
                                   name=f"eB_{b}_{qb}_{kt}")
                    nc.scalar.activation(eA[:], tA[:], AF.Exp, scale=SCALE)
                    nc.scalar.activation(eB[:], tB[:], AF.Exp, scale=SCALE)
                    for h in range(4):
                        rhs = (eA if h < 2 else eB)[:, h % 2, :]
                        nc.tensor.matmul(
                            pvp[32 * h:32 * h + 32, :],
                            V_sb[:, b * KT + kt, 32 * h:32 * h + 32],
                            rhs, start=(kt == 0), stop=(kt == KT - 1),
                            tile_position=(0, 32 * h))
                        nc.tensor.matmul(
                            dnp[32 * h:32 * h + 1, :],
                            ones_bf[:], rhs,
                            start=(kt == 0), stop=(kt == KT - 1),
                            tile_position=(0, 32 * h))
                # epilogue part 1: denominators off PSUM, reciprocal
                for h in range(4):
                    nc.vector.tensor_copy(dn_st[32 * h:32 * h + 1, :],
                                          dnp[32 * h:32 * h + 1, :])
                nc.vector.reciprocal_approx_fast(out=rec_full[:],
                                                 in_=dn_st[:])

                def epilogue():
                    recb = ps.tile([128, QB], F32, tag="sp", bufs=3,
                                   name=f"recb_{b}_{qb}")
                    nc.tensor.matmul(recb[:], eb2[:], rec_full[:],
                                     start=True, stop=True)
                    attf = work.tile([128, QB], F32, tag="attf", bufs=3,
                                     name=f"attf_{b}_{qb}")
                    nc.vector.tensor_copy(attf[:], pvp[:])
                    att = work.tile([128, QB], BF16, tag="att", bufs=3,
                                    name=f"att_{b}_{qb}")
                    nc.vector.tensor_tensor(att[:], attf[:], recb[:],
                                            ALU.mult)
                    ck = 2 * b + qb // 2
                    j0 = 4 * (qb % 2)
                    for j4 in range(4):
                        nc.sync.dma_start(a2a_in[ck][j0 + j4],
                                          att[:, 128 * j4:128 * j4 + 128])
                    if debug:
                        nc.sync.dma_start(dbg_att[b * NQB + qb], att[:])
                        nc.sync.dma_start(dbg_rec[b * NQB + qb], rec_full[:])

                pending_epi[0] = epilogue

            def a2a_exchange(ck):
                nc.gpsimd.collective_compute(
                    "AllToAll", ALU.bypass,
                    replica_groups=[list(range(N_CORES))],
                    ins=[a2a_in[ck].opt()], outs=[a2a_out[ck].opt()])

            # ---------------- phase C: out-proj + LN (per 128 rows) -------
            def outproj_load(ck):
                ab = work.tile([128, 8, 128], BF16, tag="a2asb", bufs=2,
                               name=f"ab_{ck}")
                for i in range(N_CORES):
                    nc.sync.dma_start(ab[:, i, :], a2a_out[ck][i])
                osb = work.tile([128, DIM], F32, tag="osb", bufs=2,
                                name=f"osb_{ck}")
                return ab, osb

            def outproj_mm(ck, ab, osb, nb):
                op = ps.tile([128, 512], F32, tag="sp", bufs=3,
                             name=f"op_{ck}_{nb}")
                for i in range(N_CORES):
                    nc.tensor.matmul(
                        op[:], ab[:, i, :],
                        wout_bf[:, i, nb * 512:(nb + 1) * 512],
                        start=(i == 0), stop=(i == N_CORES - 1))
                nc.vector.tensor_tensor(
                    osb[:, nb * 512:(nb + 1) * 512], op[:],
                    bout_bc[:, nb * 512:(nb + 1) * 512], ALU.add)

            def outproj_chunk(ck):
                ab, osb = outproj_load(ck)
                for nb in range(2):
                    outproj_mm(ck, ab, osb, nb)
                outproj_ln(ck, osb)

            def outproj_ln(ck, osb):
                # LayerNorm over the 1024 free dim
                stats = work.tile([128, 2, 6], F32, tag="stats", bufs=2,
                                  name=f"stats_{ck}")
                for sg in range(2):
                    nc.vector.bn_stats(out=stats[:, sg, :],
                                       in_=osb[:, sg * 512:(sg + 1) * 512])
                mv = work.tile([128, 2], F32, tag="mv", bufs=2,
                               name=f"mv_{ck}")
                nc.vector.bn_aggr(out=mv[:], in_=stats[:])
                # rstd = exp(-0.5 * ln(var + eps)) — stays in the exp/ln set
                lnv = work.tile([128, 1], F32, tag="lnv", bufs=2,
                                name=f"lnv_{ck}")
                nc.scalar.activation(out=lnv[:], in_=mv[:, 1:2], func=AF.Ln,
                                     bias=eps_sb[:], scale=1.0)
                rstd = work.tile([128, 1], F32, tag="rstd", bufs=2,
                                 name=f"rstd_{ck}")
                nc.scalar.activation(out=rstd[:], in_=lnv[:], func=AF.Exp,
                                     scale=-0.5)
                nc.vector.tensor_scalar(
                    out=osb[:], in0=osb[:], scalar1=mv[:, 0:1],
                    scalar2=rstd[:], op0=ALU.subtract, op1=ALU.mult)
                nc.vector.tensor_tensor(osb[:], osb[:], gamma_bc[:], ALU.mult)
                nc.vector.tensor_tensor(osb[:], osb[:], beta_bc[:], ALU.add)
                nc.sync.dma_start(out_d[ck * 128:(ck + 1) * 128, :], osb[:])

            # ---------------- schedule ----------------
            # per-kt emission hooks: stream the remaining projections and
            # weight loads into the attention pipeline instead of bursts
            def extra_00(kt):     # proj chunks 1..3 (rest of batch 0)
                if kt in (0, 4, 8):
                    proj_rowchunk(1 + kt // 4)

            attention_qblock(0, 0, extra_00)
            load_ln_consts()
            proj_rowchunk(4)
            attention_qblock(0, 1)
            flush_epi()
            a2a_exchange(0)
            proj_rowchunk(5)
            load_wout(0)
            load_wout(1)
            attention_qblock(0, 2)
            proj_rowchunk(6)
            load_wout(2)
            load_wout(3)
            attention_qblock(0, 3)
            flush_epi()
            a2a_exchange(1)
            proj_rowchunk(7)
            load_wout(4)
            load_wout(5)
            attention_qblock(1, 0)
            load_wout(6)
            load_wout(7)
            attention_qblock(1, 1)
            flush_epi()
            a2a_exchange(2)

            op_state = {}

            def extra_op(ck):
                def hook(kt):
                    if kt == 2:
                        op_state[ck] = outproj_load(ck)
                    elif kt == 6:
                        outproj_mm(ck, *op_state[ck], 0)
                    elif kt == 10:
                        outproj_mm(ck, *op_state[ck], 1)
                    elif kt == 14:
                        outproj_ln(ck, op_state[ck][1])
                return hook

            attention_qblock(1, 2, extra_op(0))
            attention_qblock(1, 3, extra_op(1))
            flush_epi()
            a2a_exchange(3)
            outproj_chunk(2)
            outproj_chunk(3)
            if debug:
                nc.sync.dma_start(dbg_qT, qT_sb[:].bitcast(F32))
                nc.sync.dma_start(dbg_kT, kT_sb[:].bitcast(F32))
                nc.sync.dma_start(dbg_V, V_sb[:])
                for k in range(NCK):
                    sbk = work.tile([128, 8, 128], BF16, tag="a2asb", bufs=2,
                                    name=f"dbga_{k}")
                    for i in range(N_CORES):
                        nc.sync.dma_start(sbk[:, i, :], a2a_out[k][i])
                    nc.sync.dma_start(
                        dbg_a2o[k].rearrange("c p n -> p c n"), sbk[:])

    nc.compile()
    return nc


class _Runner:
    """Compile once; run the SPMD kernel on 8 cores via PJRT repeatedly."""

    def __init__(self):
        self.nc = _build()
        import jax
        from jax.sharding import Mesh, PartitionSpec, NamedSharding
        from jax.experimental.shard_map import shard_map
        from concourse import bass2jax
        bass2jax.install_neuronx_cc_hook()

        nc = self.nc
        part_name = (nc.partition_id_tensor.name
                     if nc.partition_id_tensor else None)
        in_names, out_names, out_avals = [], [], []
        for alloc in nc.m.functions[0].allocations:
            if not isinstance(alloc, mybir.MemoryLocationSet):
                continue
            name = alloc.memorylocations[0].name
            if alloc.kind == "ExternalInput":
                if name != part_name:
                    in_names.append(name)
            elif alloc.kind == "ExternalOutput":
                out_names.append(name)
                out_avals.append(jax.core.ShapedArray(
                    tuple(alloc.tensor_shape), mybir.dt.np(alloc.dtype)))
        self.in_names = list(in_names)
        self.out_names = out_names
        self.out_avals = out_avals
        all_in_names = in_names + out_names
        if part_name is not None:
            all_in_names = all_in_names + [part_name]

        def _body(*args):
            operands = list(args)
            if part_name is not None:
                operands.append(bass2jax.partition_id_tensor())
            outs = bass2jax._bass_exec_p.bind(
                *operands, out_avals=tuple(out_avals),
                in_names=tuple(all_in_names), out_names=tuple(out_names),
                lowering_input_output_aliases=(),
                sim_require_finite=True, sim_require_nnan=True, nc=nc)
            return tuple(outs)

        devices = jax.devices()[:N_CORES]
        mesh = Mesh(np.asarray(devices), ("core",))
        self.sharding = NamedSharding(mesh, PartitionSpec("core"))
        nin = len(self.in_names) + len(out_names)
        self.fn = jax.jit(shard_map(
            _body, mesh=mesh, in_specs=(PartitionSpec("core"),) * nin,
            out_specs=(PartitionSpec("core"),) * len(out_names),
            check_rep=False))
        self.jax = jax

    def stage(self, in_maps):
        """Concatenate per-core inputs + zero outputs; device_put with the
        mesh sharding so steady-state calls skip any resharding."""
        concat = [np.concatenate([m[name] for m in in_maps], axis=0)
                  for name in self.in_names]
        zeros = [np.zeros((N_CORES * a.shape[0], *a.shape[1:]), a.dtype)
                 for a in self.out_avals]
        return [self.jax.device_put(x, self.sharding) for x in concat + zeros]

    def run_staged(self, staged):
        outs = self.fn(*staged)
        self.jax.block_until_ready(outs)
        return outs

    def run(self, in_maps):
        outs = self.run_staged(self.stage(in_maps))
        return [
            {name: np.asarray(outs[i]).reshape(
                N_CORES, *self.out_avals[i].shape)[c]
             for i, name in enumerate(self.out_names)}
            for c in range(N_CORES)
        ]


_RUNNER = None


def _get_runner():
    global _RUNNER
    if _RUNNER is None:
        _RUNNER = _Runner()
    return _RUNNER


def _make_in_maps(x, w_qkv, w_out, b_out, ln_gamma, ln_beta):
    x = np.asarray(x, dtype=np.float32)
    w_qkv = np.asarray(w_qkv, dtype=np.float32)
    w_out = np.asarray(w_out, dtype=np.float32)
    b_out = np.asarray(b_out, dtype=np.float32)
    ln_gamma = np.asarray(ln_gamma, dtype=np.float32)
    ln_beta = np.asarray(ln_beta, dtype=np.float32)

    xT = np.ascontiguousarray(x.reshape(ROWS, DIM).T)
    in_maps = []
    for c in range(N_CORES):
        h0 = HPC * c * DH
        cols = np.concatenate([
            w_qkv[:, h0:h0 + HPC * DH],
            w_qkv[:, DIM + h0:DIM + h0 + HPC * DH],
            w_qkv[:, 2 * DIM + h0:2 * DIM + h0 + HPC * DH],
        ], axis=1)
        in_maps.append({
            "xT": xT,
            "wqkv": np.ascontiguousarray(cols),
            "wout": w_out,
            "bout": b_out,
            "gamma": ln_gamma,
            "beta": ln_beta,
        })
    return in_maps


def kernel(x, w_qkv, w_out, b_out, ln_gamma, ln_beta):
    runner = _get_runner()
    in_maps = _make_in_maps(x, w_qkv, w_out, b_out, ln_gamma, ln_beta)
    results = runner.run(in_maps)
    # per-core out rows: [chunk(4), 128]; global row = 1024*ck + 128*c + r
    full = np.empty((ROWS, DIM), dtype=np.float32)
    for c in range(N_CORES):
        o = results[c]["out"]
        for ck in range(NCK):
            r0 = 1024 * ck + 128 * c
            full[r0:r0 + 128] = o[ck * 128:(ck + 1) * 128]
    return full.reshape(B, N, DIM)



# revision 8
# speedup vs baseline: 1.0684x; 1.0684x over previous
"""Fused multi-head attention block (QKV proj + softmax attention + out-proj
+ LayerNorm) for Trainium2, sharded over 8 NeuronCores.

Sharding: tensor-parallel over heads. Core c owns heads [4c, 4c+4).

v2 design (vs the 508us baseline):
  - Denominator matmuls eliminated: PV lhsT is [V_h | ones] (M=33), so the
    per-head softmax denominator accumulates on PSUM partition 32/96 of the
    same stream that computes P@V.  Saves ~1/3 of attention PE cycles.
  - exp split across two engines so the Scalar engine stops pacing the PE
    (which kept HAM-throttling to half clock): ACT does exact exp for heads
    0-1 (eA); DVE does a Schraudolph bf16 exp (t = S*a+b -> int16 ->
    bitcast bf16, ~3% rel err, row-common part cancels in softmax) for
    heads 2-3 (eB).
  - Epilogue restructured: PSUM drained fast (ACT copies dn rows, DVE
    copies att rows), normalization (reciprocal + eb2 broadcast matmul +
    multiply) deferred; the final multiply runs on GPSIMD (SBUF-only).
  - Collectives issued from the Sync queue; xt DMAs alternate
    sync/gpsimd; startup DMAs (wqkv + first x chunk) split across 5 engine
    queues so the first matmul starts at ~16us instead of ~43us.

dtypes: fp32 storage; QKV and S^T matmuls run as float32r; exp(S^T), PV
and out-proj use bf16.  LayerNorm rstd uses exp(-0.5*ln(var+eps)) so the
Scalar engine keeps a single activation table set loaded.
"""
import sys

for _p in ("/opt/trn_rl_repo", "/root/.axon_site/_ro/trn_rl_repo"):
    if _p not in sys.path:
        sys.path.insert(0, _p)

import numpy as np

import concourse.bass as bass
import concourse.tile as tile
from concourse import bacc, mybir
from concourse.masks import make_identity

F32 = mybir.dt.float32
F32R = mybir.dt.float32r
BF16 = mybir.dt.bfloat16
I16 = mybir.dt.int16
AF = mybir.ActivationFunctionType
ALU = mybir.AluOpType

N_CORES = 8
B, N, DIM = 2, 2048, 1024
HEADS, DH = 32, 32           # 32 heads x 32 dim/head
HPC = HEADS // N_CORES       # 4 heads per core
ROWS = B * N                 # 4096 global rows
SCALE = DH ** -0.5
EPS = 1e-6
KT = N // 128                # 16 key tiles per batch
QB = 512                     # q-block width
NQB = N // QB                # 4 q-blocks per batch
RC = 512                     # projection row-chunk
NRC = ROWS // RC             # 16 row chunks
NCK = 4                      # a2a chunks; chunk k = global rows [1024k, 1024k+1024)
                             # core c owns rows 1024k + 128c .. +128 of each chunk

# Schraudolph exp in bf16: exp(x) ~ bitcast_bf16(int16(x * A + B))
# (hardware rounds to nearest on the f32->int16 convert).
SCH_A = (2.0 ** 7) / np.log(2.0) * SCALE   # folds the 1/sqrt(dh) scale in
SCH_B = 127.0 * 2 ** 7 - 4.5


def _build():
    nc = bacc.Bacc("TRN2", target_bir_lowering=False, debug=False,
                   num_devices=N_CORES)

    xT_d = nc.dram_tensor("xT", [DIM, ROWS], F32R, kind="ExternalInput").ap()
    wqkv_d = nc.dram_tensor("wqkv", [DIM, 3 * HPC * DH], F32R,
                            kind="ExternalInput").ap()
    wout_d = nc.dram_tensor("wout", [DIM, DIM], F32, kind="ExternalInput").ap()
    bout_d = nc.dram_tensor("bout", [DIM], F32, kind="ExternalInput").ap()
    gamma_d = nc.dram_tensor("gamma", [DIM], F32, kind="ExternalInput").ap()
    beta_d = nc.dram_tensor("beta", [DIM], F32, kind="ExternalInput").ap()
    # rows: [chunk(4), 128]; global row = 1024*ck + 128*core + r
    out_d = nc.dram_tensor("out", [NCK * 128, DIM], F32,
                           kind="ExternalOutput").ap()

    with tile.TileContext(nc) as tc:
        with (
            tc.tile_pool(name="const", bufs=1) as const,
            tc.tile_pool(name="work", bufs=1) as work,
            tc.tile_pool(name="ps", bufs=1, space="PSUM") as ps,
            tc.tile_pool(name="dram", bufs=1, space="DRAM") as dram,
        ):
            # ---------------- constants / weights ----------------
            # wqkv split across two queues to shorten the startup window
            wqkv_sb = const.tile([128, 8, 3 * HPC * DH], F32R)
            wqkv_r = wqkv_d.rearrange("(kc p) m -> p kc m", p=128)
            for kc in range(4):
                nc.scalar.dma_start(wqkv_sb[:, 2 * kc:2 * kc + 2, :],
                                    wqkv_r[:, 2 * kc:2 * kc + 2, :])
            ident = const.tile([128, 128], F32)
            make_identity(nc, ident[:])
            eps_sb = const.tile([128, 1], F32)
            nc.vector.memset(eps_sb[:], EPS)
            # slot-broadcast matrix: contracting eb2s against a [128, q]
            # tile whose row 32 / 96 holds the per-head reciprocal
            # denominator broadcasts row 32 -> partitions [0,32) and row
            # 96 -> partitions [64,96); all other output rows get 0.
            eb2s = const.tile([128, 128], F32)
            nc.vector.memset(eb2s[:], 0.0)
            nc.vector.memset(eb2s[32:33, 0:32], 1.0)
            nc.vector.memset(eb2s[96:97, 64:96], 1.0)
            # per-slot denominator staging rows (junk rows stay 1.0 so the
            # full-tile fast reciprocal stays finite)
            dn_st = [const.tile([128, QB], F32, name=f"dnst_{j}")
                     for j in range(2)]
            for j in range(2):
                nc.vector.memset(dn_st[j][:], 1.0)
            # warm the ACT table set (ln first so the shared
            # natural_log_exp_and_others set is chosen, then exp)
            scr = const.tile([128, 1], F32)
            nc.scalar.activation(out=scr[:], in_=eps_sb[:], func=AF.Ln,
                                 bias=eps_sb[:], scale=1.0)
            nc.scalar.activation(out=scr[:], in_=eps_sb[:], func=AF.Exp,
                                 scale=1.0)
            # row-broadcast vectors [128, 1024]; DMAs deferred past the
            # startup window so they don't queue ahead of the xT loads
            bout_bc = const.tile([128, DIM], F32)
            gamma_bc = const.tile([128, DIM], F32)
            beta_bc = const.tile([128, DIM], F32)

            def load_ln_consts():
                for bc, src_d in ((bout_bc, bout_d), (gamma_bc, gamma_d),
                                  (beta_bc, beta_d)):
                    nc.sync.dma_start(out=bc[:], in_=bass.AP(
                        tensor=src_d.tensor, offset=src_d.offset,
                        ap=[[0, 128], [1, DIM]]))
            # w_out -> bf16 [128, 8, 1024]
            wout_bf = const.tile([128, 8, DIM], BF16)

            # ---------------- persistent activations ----------------
            qT_sb = const.tile([128, ROWS], F32R)   # 4h x 32d on partitions
            kT_sb = const.tile([128, ROWS], F32R)
            # V with a ones column per head: [key%128, ktile, head, 33]
            V_sb = const.tile([128, 2 * KT, HPC, DH + 1], BF16)
            nc.vector.memset(V_sb[:, :, :, DH:DH + 1], 1.0)

            # ---------------- dram bounce buffers ----------------
            a2a_in = [dram.tile([N_CORES, 128, 128], BF16, name=f"a2ai_{k}")
                      for k in range(NCK)]
            a2a_out = [dram.tile([N_CORES, 128, 128], BF16, name=f"a2ao_{k}")
                       for k in range(NCK)]

            # ---------------- phase A: projections ----------------
            def proj_rowchunk(rc, split=False):
                xt = work.tile([128, 8, RC], F32R, tag="xt", bufs=3,
                               name=f"xt_{rc}")
                src = (xT_d[:, rc * RC:(rc + 1) * RC]
                       .rearrange("(kc p) n -> p kc n", p=128))
                if split:
                    # startup: split into 4 DMAs (parallel hw queues) on
                    # sync + 2 on gpsimd
                    for kc in range(3):
                        nc.sync.dma_start(xt[:, 2 * kc:2 * kc + 2, :],
                                          src[:, 2 * kc:2 * kc + 2, :])
                    nc.gpsimd.dma_start(xt[:, 6:8, :], src[:, 6:8, :])
                else:
                    # keep the gpsimd queue free for collectives: even
                    # chunks on sync, odd chunks on the scalar hwdge
                    dma_eng = nc.sync if rc % 2 == 0 else nc.scalar
                    dma_eng.dma_start(xt[:], src)
                for name, mofs, dst in (("q", 0, qT_sb), ("k", 128, kT_sb)):
                    pp = ps.tile([128, RC], F32, tag="sp", bufs=3,
                                 name=f"pp_{name}_{rc}")
                    for kc in range(8):
                        nc.tensor.matmul(
                            pp[:], wqkv_sb[:, kc, mofs:mofs + 128],
                            xt[:, kc, :], start=(kc == 0), stop=(kc == 7))
                    nc.vector.tensor_copy(dst[:, rc * RC:(rc + 1) * RC], pp[:])
                # v: project (vT layout), cast bf16, DMA-transpose into V_sb
                pv_ = ps.tile([128, RC], F32, tag="sp", bufs=3,
                               name=f"pp_v_{rc}")
                for kc in range(8):
                    nc.tensor.matmul(
                        pv_[:], wqkv_sb[:, kc, 256:384], xt[:, kc, :],
                        start=(kc == 0), stop=(kc == 7))
                vt = work.tile([128, RC], F32, tag="vt", bufs=2,
                               name=f"vt_{rc}")
                nc.vector.tensor_copy(vt[:], pv_[:])
                for i in range(RC // 128):
                    tp = ps.tile([128, 128], F32, tag="sp", bufs=3,
                                 name=f"tp_{rc}_{i}")
                    nc.tensor.matmul(
                        tp[:], vt[:, i * 128:(i + 1) * 128], ident[:],
                        is_transpose=True, start=True, stop=True)
                    kt_abs = rc * (RC // 128) + i
                    nc.vector.tensor_copy(
                        V_sb[:, kt_abs, :, 0:DH],
                        tp[:].rearrange("p (h d) -> p h d", h=HPC))

            def load_wout(j):
                st = work.tile([128, DIM], F32, tag="wstage", bufs=2,
                               name=f"wst_{j}")
                nc.sync.dma_start(st[:], wout_d[j * 128:(j + 1) * 128, :])
                nc.scalar.copy(wout_bf[:, j, :], st[:])

            proj_rowchunk(0, split=True)    # enough rows to start qb0

            # ---------------- phase B: attention ----------------
            # Software-pipelined per q-block: S two key-tiles ahead; ACT
            # exp paces heads 0-1, DVE Schraudolph paces heads 2-3; PV
            # (with fused denominator column) trails by one tile.
            def emit_S(b, qb, kt):
                q0 = b * N + qb * QB
                k0 = b * N + kt * 128
                tA = ps.tile([128, 2, QB], F32, tag="sp", bufs=3,
                             name=f"sA_{b}_{qb}_{kt}")
                tB = ps.tile([128, 2, QB], F32, tag="sp", bufs=3,
                             name=f"sB_{b}_{qb}_{kt}")
                for h in range(4):
                    t = tA if h < 2 else tB
                    nc.tensor.matmul(
                        t[:, h % 2, :],
                        kT_sb[32 * h:32 * h + 32, k0:k0 + 128],
                        qT_sb[32 * h:32 * h + 32, q0:q0 + QB],
                        start=True, stop=True, tile_position=(32 * h, 0))
                return tA, tB

            pending_epi = [None]

            def flush_epi():
                if pending_epi[0] is not None:
                    fn = pending_epi[0]
                    pending_epi[0] = None
                    fn()

            def attention_qblock(b, qb, extra=None):
                # PV+dn accumulator: slot j holds heads {j, j+? } ->
                # head h: partitions [64*(h%2), +33), bank h//2.
                pvp = ps.tile([128, 2, QB], F32, tag="pv", name=f"pv_{b}_{qb}")
                s_tiles = {0: emit_S(b, qb, 0), 1: emit_S(b, qb, 1)}
                flush_epi()   # previous q-block's tail, behind our first S
                for kt in range(KT):
                    if extra is not None:
                        extra(kt)
                    if kt + 2 < KT:
                        s_tiles[kt + 2] = emit_S(b, qb, kt + 2)
                    tA, tB = s_tiles.pop(kt)
                    eA = work.tile([128, 2, QB], BF16, tag="expt", bufs=8,
                                   name=f"eA_{b}_{qb}_{kt}")
                    eB = work.tile([128, 2, QB], BF16, tag="expt", bufs=8,
                                   name=f"eB_{b}_{qb}_{kt}")
                    nc.scalar.activation(eA[:], tA[:], AF.Exp, scale=SCALE)
                    nc.vector.tensor_scalar(
                        out=eB[:].bitcast(I16), in0=tB[:],
                        scalar1=float(SCH_A), scalar2=float(SCH_B),
                        op0=ALU.mult, op1=ALU.add)
                    for h in range(4):
                        rhs = (eA if h < 2 else eB)[:, h % 2, :]
                        p0 = 64 * (h % 2)
                        nc.tensor.matmul(
                            pvp[p0:p0 + DH + 1, h // 2, :],
                            V_sb[:, b * KT + kt, h, :],
                            rhs, start=(kt == 0), stop=(kt == KT - 1),
                            tile_position=(0, p0))
                # drain PSUM fast: dn rows via ACT, att rows via DVE
                attf = [work.tile([128, QB], BF16, tag="attf", bufs=4,
                                  name=f"attf_{b}_{qb}_{j}")
                        for j in range(2)]
                for j in range(2):
                    nc.scalar.copy(dn_st[j][32:33, :], pvp[32:33, j, :])
                    nc.scalar.copy(dn_st[j][96:97, :], pvp[96:97, j, :])
                    nc.vector.tensor_copy(attf[j][0:32, :], pvp[0:32, j, :])
                    nc.vector.tensor_copy(attf[j][64:96, :], pvp[64:96, j, :])

                def epilogue():
                    ck = 2 * b + qb // 2
                    j0 = 4 * (qb % 2)
                    for j in range(2):
                        rec = work.tile([128, QB], F32, tag="rec", bufs=4,
                                        name=f"rec_{b}_{qb}_{j}")
                        nc.vector.reciprocal_approx_fast(out=rec[:],
                                                         in_=dn_st[j][:])
                        recb = ps.tile([128, QB], F32, tag="sp", bufs=3,
                                       name=f"recb_{b}_{qb}_{j}")
                        nc.tensor.matmul(recb[:], eb2s[:], rec[:],
                                         start=True, stop=True)
                        recb_sb = work.tile([128, QB], BF16, tag="recs",
                                            bufs=4, name=f"recs_{b}_{qb}_{j}")
                        nc.scalar.copy(recb_sb[:], recb[:])
                        att = work.tile([128, QB], BF16, tag="att", bufs=4,
                                        name=f"att_{b}_{qb}_{j}")
                        for g in range(2):
                            sl = slice(64 * g, 64 * g + 32)
                            nc.gpsimd.tensor_tensor(
                                att[sl, :], attf[j][sl, :], recb_sb[sl, :],
                                ALU.mult)
                        # gather DMA: partitions {0-31, 64-95} -> block rows
                        # [64j+32g, +32) of each of the 4 dram blocks
                        for g in range(2):
                            r0 = 64 * j + 32 * g
                            nc.sync.dma_start(
                                a2a_in[ck][j0:j0 + 4, r0:r0 + 32, :]
                                .rearrange("j p q -> p j q"),
                                att[64 * g:64 * g + 32, :]
                                .rearrange("p (j q) -> p j q", j=4))

                pending_epi[0] = epilogue

            def a2a_exchange(ck):
                nc.gpsimd.collective_compute(
                    "AllToAll", ALU.bypass,
                    replica_groups=[list(range(N_CORES))],
                    ins=[a2a_in[ck].opt()], outs=[a2a_out[ck].opt()])

            # ---------------- phase C: out-proj + LN (per 128 rows) -------
            def outproj_load(ck):
                ab = work.tile([128, 8, 128], BF16, tag="a2asb", bufs=2,
                               name=f"ab_{ck}")
                for i in range(N_CORES):
                    nc.sync.dma_start(ab[:, i, :], a2a_out[ck][i])
                osb = work.tile([128, DIM], F32, tag="osb", bufs=2,
                                name=f"osb_{ck}")
                return ab, osb

            def outproj_mm(ck, ab, osb, nb):
                op = ps.tile([128, 512], F32, tag="sp", bufs=3,
                             name=f"op_{ck}_{nb}")
                for i in range(N_CORES):
                    nc.tensor.matmul(
                        op[:], ab[:, i, :],
                        wout_bf[:, i, nb * 512:(nb + 1) * 512],
                        start=(i == 0), stop=(i == N_CORES - 1))
                nc.vector.tensor_tensor(
                    osb[:, nb * 512:(nb + 1) * 512], op[:],
                    bout_bc[:, nb * 512:(nb + 1) * 512], ALU.add)

            def outproj_chunk(ck):
                ab, osb = outproj_load(ck)
                for nb in range(2):
                    outproj_mm(ck, ab, osb, nb)
                outproj_ln(ck, osb)

            def outproj_ln(ck, osb):
                # LayerNorm over the 1024 free dim
                stats = work.tile([128, 2, 6], F32, tag="stats", bufs=2,
                                  name=f"stats_{ck}")
                for sg in range(2):
                    nc.vector.bn_stats(out=stats[:, sg, :],
                                       in_=osb[:, sg * 512:(sg + 1) * 512])
                mv = work.tile([128, 2], F32, tag="mv", bufs=2,
                               name=f"mv_{ck}")
                nc.vector.bn_aggr(out=mv[:], in_=stats[:])
                # rstd = exp(-0.5 * ln(var + eps)) — stays in the exp/ln set
                lnv = work.tile([128, 1], F32, tag="lnv", bufs=2,
                                name=f"lnv_{ck}")
                nc.scalar.activation(out=lnv[:], in_=mv[:, 1:2], func=AF.Ln,
                                     bias=eps_sb[:], scale=1.0)
                rstd = work.tile([128, 1], F32, tag="rstd", bufs=2,
                                 name=f"rstd_{ck}")
                nc.scalar.activation(out=rstd[:], in_=lnv[:], func=AF.Exp,
                                     scale=-0.5)
                nc.vector.tensor_scalar(
                    out=osb[:], in0=osb[:], scalar1=mv[:, 0:1],
                    scalar2=rstd[:], op0=ALU.subtract, op1=ALU.mult)
                nc.gpsimd.tensor_tensor(osb[:], osb[:], gamma_bc[:], ALU.mult)
                nc.gpsimd.tensor_tensor(osb[:], osb[:], beta_bc[:], ALU.add)
                nc.sync.dma_start(out_d[ck * 128:(ck + 1) * 128, :], osb[:])

            # ---------------- schedule ----------------
            # per-kt emission hooks: stream the remaining projections and
            # weight loads into the attention pipeline instead of bursts
            def extra_00(kt):     # proj chunks 1..3 (rest of batch 0)
                if kt in (0, 4, 8):
                    proj_rowchunk(1 + kt // 4)

            attention_qblock(0, 0, extra_00)
            load_ln_consts()
            proj_rowchunk(4)
            attention_qblock(0, 1)
            flush_epi()
            a2a_exchange(0)
            proj_rowchunk(5)
            load_wout(0)
            load_wout(1)
            attention_qblock(0, 2)
            proj_rowchunk(6)
            load_wout(2)
            load_wout(3)
            attention_qblock(0, 3)
            flush_epi()
            a2a_exchange(1)
            proj_rowchunk(7)
            load_wout(4)
            load_wout(5)
            attention_qblock(1, 0)
            load_wout(6)
            load_wout(7)
            attention_qblock(1, 1)
            flush_epi()
            a2a_exchange(2)

            op_state = {}

            def extra_op(ck):
                def hook(kt):
                    if kt == 2:
                        op_state[ck] = outproj_load(ck)
                    elif kt == 6:
                        outproj_mm(ck, *op_state[ck], 0)
                    elif kt == 10:
                        outproj_mm(ck, *op_state[ck], 1)
                    elif kt == 14:
                        outproj_ln(ck, op_state[ck][1])
                return hook

            attention_qblock(1, 2, extra_op(0))
            attention_qblock(1, 3, extra_op(1))
            flush_epi()
            a2a_exchange(3)
            outproj_chunk(2)
            outproj_chunk(3)

    nc.compile()
    return nc


class _Runner:
    """Compile once; run the SPMD kernel on 8 cores via PJRT repeatedly."""

    def __init__(self):
        self.nc = _build()
        import jax
        from jax.sharding import Mesh, PartitionSpec, NamedSharding
        from jax.experimental.shard_map import shard_map
        from concourse import bass2jax
        bass2jax.install_neuronx_cc_hook()

        nc = self.nc
        part_name = (nc.partition_id_tensor.name
                     if nc.partition_id_tensor else None)
        in_names, out_names, out_avals = [], [], []
        for alloc in nc.m.functions[0].allocations:
            if not isinstance(alloc, mybir.MemoryLocationSet):
                continue
            name = alloc.memorylocations[0].name
            if alloc.kind == "ExternalInput":
                if name != part_name:
                    in_names.append(name)
            elif alloc.kind == "ExternalOutput":
                out_names.append(name)
                out_avals.append(jax.core.ShapedArray(
                    tuple(alloc.tensor_shape), mybir.dt.np(alloc.dtype)))
        self.in_names = list(in_names)
        self.out_names = out_names
        self.out_avals = out_avals
        all_in_names = in_names + out_names
        if part_name is not None:
            all_in_names = all_in_names + [part_name]

        def _body(*args):
            operands = list(args)
            if part_name is not None:
                operands.append(bass2jax.partition_id_tensor())
            outs = bass2jax._bass_exec_p.bind(
                *operands, out_avals=tuple(out_avals),
                in_names=tuple(all_in_names), out_names=tuple(out_names),
                lowering_input_output_aliases=(),
                sim_require_finite=True, sim_require_nnan=True, nc=nc)
            return tuple(outs)

        devices = jax.devices()[:N_CORES]
        mesh = Mesh(np.asarray(devices), ("core",))
        self.sharding = NamedSharding(mesh, PartitionSpec("core"))
        nin = len(self.in_names) + len(out_names)
        self.fn = jax.jit(shard_map(
            _body, mesh=mesh, in_specs=(PartitionSpec("core"),) * nin,
            out_specs=(PartitionSpec("core"),) * len(out_names),
            check_rep=False))
        self.jax = jax

    def stage(self, in_maps):
        """Concatenate per-core inputs + zero outputs; device_put with the
        mesh sharding so steady-state calls skip any resharding."""
        concat = [np.concatenate([m[name] for m in in_maps], axis=0)
                  for name in self.in_names]
        zeros = [np.zeros((N_CORES * a.shape[0], *a.shape[1:]), a.dtype)
                 for a in self.out_avals]
        return [self.jax.device_put(x, self.sharding) for x in concat + zeros]

    def run_staged(self, staged):
        outs = self.fn(*staged)
        self.jax.block_until_ready(outs)
        return outs

    def run(self, in_maps):
        outs = self.run_staged(self.stage(in_maps))
        return [
            {name: np.asarray(outs[i]).reshape(
                N_CORES, *self.out_avals[i].shape)[c]
             for i, name in enumerate(self.out_names)}
            for c in range(N_CORES)
        ]


_RUNNER = None


def _get_runner():
    global _RUNNER
    if _RUNNER is None:
        _RUNNER = _Runner()
    return _RUNNER


def _make_in_maps(x, w_qkv, w_out, b_out, ln_gamma, ln_beta):
    x = np.asarray(x, dtype=np.float32)
    w_qkv = np.asarray(w_qkv, dtype=np.float32)
    w_out = np.asarray(w_out, dtype=np.float32)
    b_out = np.asarray(b_out, dtype=np.float32)
    ln_gamma = np.asarray(ln_gamma, dtype=np.float32)
    ln_beta = np.asarray(ln_beta, dtype=np.float32)

    xT = np.ascontiguousarray(x.reshape(ROWS, DIM).T)
    in_maps = []
    for c in range(N_CORES):
        h0 = HPC * c * DH
        cols = np.concatenate([
            w_qkv[:, h0:h0 + HPC * DH],
            w_qkv[:, DIM + h0:DIM + h0 + HPC * DH],
            w_qkv[:, 2 * DIM + h0:2 * DIM + h0 + HPC * DH],
        ], axis=1)
        in_maps.append({
            "xT": xT,
            "wqkv": np.ascontiguousarray(cols),
            "wout": w_out,
            "bout": b_out,
            "gamma": ln_gamma,
            "beta": ln_beta,
        })
    return in_maps


def kernel(x, w_qkv, w_out, b_out, ln_gamma, ln_beta):
    runner = _get_runner()
    in_maps = _make_in_maps(x, w_qkv, w_out, b_out, ln_gamma, ln_beta)
    results = runner.run(in_maps)
    # per-core out rows: [chunk(4), 128]; global row = 1024*ck + 128*c + r
    full = np.empty((ROWS, DIM), dtype=np.float32)
    for c in range(N_CORES):
        o = results[c]["out"]
        for ck in range(NCK):
            r0 = 1024 * ck + 128 * c
            full[r0:r0 + 128] = o[ck * 128:(ck + 1) * 128]
    return full.reshape(B, N, DIM)


# revision 13
# speedup vs baseline: 1.0710x; 1.0024x over previous
"""Fused multi-head attention block (QKV proj + softmax attention + out-proj
+ LayerNorm) for Trainium2, sharded over 8 NeuronCores.

Sharding: tensor-parallel over heads. Core c owns heads [4c, 4c+4).

v2 design (vs the 508us baseline):
  - Denominator matmuls eliminated: PV lhsT is [V_h | ones] (M=33), so the
    per-head softmax denominator accumulates on PSUM partition 32/96 of the
    same stream that computes P@V.  Saves ~1/3 of attention PE cycles.
  - exp split across two engines so the Scalar engine stops pacing the PE
    (which kept HAM-throttling to half clock): ACT does exact exp for heads
    0-1 (eA); DVE does a Schraudolph bf16 exp (t = S*a+b -> int16 ->
    bitcast bf16, ~3% rel err, row-common part cancels in softmax) for
    heads 2-3 (eB).
  - Epilogue restructured: PSUM drained fast (ACT copies dn rows, DVE
    copies att rows), normalization (reciprocal + eb2 broadcast matmul +
    multiply) deferred; the final multiply runs on GPSIMD (SBUF-only).
  - Collectives issued from the Sync queue; xt DMAs alternate
    sync/gpsimd; startup DMAs (wqkv + first x chunk) split across 5 engine
    queues so the first matmul starts at ~16us instead of ~43us.

dtypes: fp32 storage; QKV and S^T matmuls run as float32r; exp(S^T), PV
and out-proj use bf16.  LayerNorm rstd uses exp(-0.5*ln(var+eps)) so the
Scalar engine keeps a single activation table set loaded.
"""
import sys

for _p in ("/opt/trn_rl_repo", "/root/.axon_site/_ro/trn_rl_repo"):
    if _p not in sys.path:
        sys.path.insert(0, _p)

import numpy as np

import concourse.bass as bass
import concourse.tile as tile
from concourse import bacc, mybir
from concourse.masks import make_identity

F32 = mybir.dt.float32
F32R = mybir.dt.float32r
BF16 = mybir.dt.bfloat16
I16 = mybir.dt.int16
AF = mybir.ActivationFunctionType
ALU = mybir.AluOpType

N_CORES = 8
B, N, DIM = 2, 2048, 1024
HEADS, DH = 32, 32           # 32 heads x 32 dim/head
HPC = HEADS // N_CORES       # 4 heads per core
ROWS = B * N                 # 4096 global rows
SCALE = DH ** -0.5
EPS = 1e-6
KT = N // 128                # 16 key tiles per batch
QB = 512                     # q-block width
NQB = N // QB                # 4 q-blocks per batch
RC = 512                     # projection row-chunk
NRC = ROWS // RC             # 16 row chunks
NCK = 4                      # a2a chunks; chunk k = global rows [1024k, 1024k+1024)
                             # core c owns rows 1024k + 128c .. +128 of each chunk

# Schraudolph exp in bf16: exp(x) ~ bitcast_bf16(int16(x * A + B))
# (hardware rounds to nearest on the f32->int16 convert).
SCH_A = (2.0 ** 7) / np.log(2.0) * SCALE   # folds the 1/sqrt(dh) scale in
SCH_B = 127.0 * 2 ** 7 - 4.5


def _build():
    nc = bacc.Bacc("TRN2", target_bir_lowering=False, debug=False,
                   num_devices=N_CORES)

    xT_d = nc.dram_tensor("xT", [DIM, ROWS], F32R, kind="ExternalInput").ap()
    wqkv_d = nc.dram_tensor("wqkv", [DIM, 3 * HPC * DH], F32R,
                            kind="ExternalInput").ap()
    wout_d = nc.dram_tensor("wout", [DIM, DIM], F32, kind="ExternalInput").ap()
    bout_d = nc.dram_tensor("bout", [DIM], F32, kind="ExternalInput").ap()
    gamma_d = nc.dram_tensor("gamma", [DIM], F32, kind="ExternalInput").ap()
    beta_d = nc.dram_tensor("beta", [DIM], F32, kind="ExternalInput").ap()
    # rows: [chunk(4), 128]; global row = 1024*ck + 128*core + r
    out_d = nc.dram_tensor("out", [NCK * 128, DIM], F32,
                           kind="ExternalOutput").ap()

    with tile.TileContext(nc) as tc:
        with (
            tc.tile_pool(name="const", bufs=1) as const,
            tc.tile_pool(name="work", bufs=1) as work,
            tc.tile_pool(name="ps", bufs=1, space="PSUM") as ps,
            tc.tile_pool(name="dram", bufs=1, space="DRAM") as dram,
        ):
            # ---------------- constants / weights ----------------
            # wqkv split across two queues to shorten the startup window
            wqkv_sb = const.tile([128, 8, 3 * HPC * DH], F32R)
            wqkv_r = wqkv_d.rearrange("(kc p) m -> p kc m", p=128)
            for kc in range(4):
                nc.scalar.dma_start(wqkv_sb[:, 2 * kc:2 * kc + 2, :],
                                    wqkv_r[:, 2 * kc:2 * kc + 2, :])
            ident = const.tile([128, 128], F32)
            make_identity(nc, ident[:])
            eps_sb = const.tile([128, 1], F32)
            nc.vector.memset(eps_sb[:], EPS)
            # slot-broadcast matrix: contracting eb2s against a [128, q]
            # tile whose row 32 / 96 holds the per-head reciprocal
            # denominator broadcasts row 32 -> partitions [0,32) and row
            # 96 -> partitions [64,96); all other output rows get 0.
            eb2s = const.tile([128, 128], BF16)
            nc.vector.memset(eb2s[:], 0.0)
            nc.vector.memset(eb2s[32:33, 0:32], 1.0)
            nc.vector.memset(eb2s[96:97, 64:96], 1.0)
            # denominator staging rows, both slots in one tile (junk rows
            # stay 1.0 so the full-tile fast reciprocal stays finite)
            dn_st = const.tile([128, 2, QB], F32)
            nc.vector.memset(dn_st[:], 1.0)
            # warm the ACT table set (ln first so the shared
            # natural_log_exp_and_others set is chosen, then exp)
            scr = const.tile([128, 1], F32)
            nc.scalar.activation(out=scr[:], in_=eps_sb[:], func=AF.Ln,
                                 bias=eps_sb[:], scale=1.0)
            nc.scalar.activation(out=scr[:], in_=eps_sb[:], func=AF.Exp,
                                 scale=1.0)
            # row-broadcast vectors [128, 1024]; DMAs deferred past the
            # startup window so they don't queue ahead of the xT loads
            bout_bc = const.tile([128, DIM], F32)
            gamma_bc = const.tile([128, DIM], F32)
            beta_bc = const.tile([128, DIM], F32)

            def load_ln_consts():
                for bc, src_d in ((bout_bc, bout_d), (gamma_bc, gamma_d),
                                  (beta_bc, beta_d)):
                    nc.sync.dma_start(out=bc[:], in_=bass.AP(
                        tensor=src_d.tensor, offset=src_d.offset,
                        ap=[[0, 128], [1, DIM]]))
            # w_out -> bf16 [128, 8, 1024]
            wout_bf = const.tile([128, 8, DIM], BF16)

            # ---------------- persistent activations ----------------
            qT_sb = const.tile([128, ROWS], F32R)   # 4h x 32d on partitions
            kT_sb = const.tile([128, ROWS], F32R)
            # V with a ones column per head: [key%128, ktile, head, 33]
            V_sb = const.tile([128, 2 * KT, HPC, DH + 1], BF16)
            nc.vector.memset(V_sb[:, :, :, DH:DH + 1], 1.0)

            # ---------------- dram bounce buffers ----------------
            a2a_in = [dram.tile([N_CORES, 128, 128], BF16, name=f"a2ai_{k}")
                      for k in range(NCK)]
            a2a_out = [dram.tile([N_CORES, 128, 128], BF16, name=f"a2ao_{k}")
                       for k in range(NCK)]

            # ---------------- phase A: projections ----------------
            def proj_rowchunk(rc, split=False):
                xt = work.tile([128, 8, RC], F32R, tag="xt", bufs=3,
                               name=f"xt_{rc}")
                src = (xT_d[:, rc * RC:(rc + 1) * RC]
                       .rearrange("(kc p) n -> p kc n", p=128))
                if split:
                    # startup: split into 4 DMAs (parallel hw queues) on
                    # sync + 2 on gpsimd
                    for kc in range(3):
                        nc.sync.dma_start(xt[:, 2 * kc:2 * kc + 2, :],
                                          src[:, 2 * kc:2 * kc + 2, :])
                    nc.gpsimd.dma_start(xt[:, 6:8, :], src[:, 6:8, :])
                else:
                    # keep the gpsimd queue free for collectives: even
                    # chunks on sync, odd chunks on the scalar hwdge
                    dma_eng = nc.sync if rc % 2 == 0 else nc.scalar
                    dma_eng.dma_start(xt[:], src)
                for name, mofs, dst in (("q", 0, qT_sb), ("k", 128, kT_sb)):
                    pp = ps.tile([128, RC], F32, tag="sp", bufs=3,
                                 name=f"pp_{name}_{rc}")
                    for kc in range(8):
                        nc.tensor.matmul(
                            pp[:], wqkv_sb[:, kc, mofs:mofs + 128],
                            xt[:, kc, :], start=(kc == 0), stop=(kc == 7))
                    nc.vector.tensor_copy(dst[:, rc * RC:(rc + 1) * RC], pp[:])
                # v: project (vT layout), cast bf16, DMA-transpose into V_sb
                pv_ = ps.tile([128, RC], F32, tag="sp", bufs=3,
                               name=f"pp_v_{rc}")
                for kc in range(8):
                    nc.tensor.matmul(
                        pv_[:], wqkv_sb[:, kc, 256:384], xt[:, kc, :],
                        start=(kc == 0), stop=(kc == 7))
                vt = work.tile([128, RC], F32, tag="vt", bufs=2,
                               name=f"vt_{rc}")
                nc.vector.tensor_copy(vt[:], pv_[:])
                for i in range(RC // 128):
                    tp = ps.tile([128, 128], F32, tag="sp", bufs=3,
                                 name=f"tp_{rc}_{i}")
                    nc.tensor.matmul(
                        tp[:], vt[:, i * 128:(i + 1) * 128], ident[:],
                        is_transpose=True, start=True, stop=True)
                    kt_abs = rc * (RC // 128) + i
                    nc.vector.tensor_copy(
                        V_sb[:, kt_abs, :, 0:DH],
                        tp[:].rearrange("p (h d) -> p h d", h=HPC))

            def load_wout(j):
                st = work.tile([128, DIM], F32, tag="wstage", bufs=2,
                               name=f"wst_{j}")
                nc.sync.dma_start(st[:], wout_d[j * 128:(j + 1) * 128, :])
                nc.scalar.copy(wout_bf[:, j, :], st[:])

            proj_rowchunk(0, split=True)    # enough rows to start qb0

            # ---------------- phase B: attention ----------------
            # Software-pipelined per q-block: S two key-tiles ahead; ACT
            # exp paces heads 0-1, DVE Schraudolph paces heads 2-3; PV
            # (with fused denominator column) trails by one tile.
            def emit_S(b, qb, kt):
                q0 = b * N + qb * QB
                k0 = b * N + kt * 128
                tA = ps.tile([128, 2, QB], F32, tag="sp", bufs=3,
                             name=f"sA_{b}_{qb}_{kt}")
                tB = ps.tile([128, 2, QB], F32, tag="sp", bufs=3,
                             name=f"sB_{b}_{qb}_{kt}")
                for h in range(4):
                    t = tA if h < 2 else tB
                    nc.tensor.matmul(
                        t[:, h % 2, :],
                        kT_sb[32 * h:32 * h + 32, k0:k0 + 128],
                        qT_sb[32 * h:32 * h + 32, q0:q0 + QB],
                        start=True, stop=True, tile_position=(32 * h, 0))
                return tA, tB

            pending_epi = [None]

            def flush_epi():
                if pending_epi[0] is not None:
                    fn = pending_epi[0]
                    pending_epi[0] = None
                    fn()

            def attention_qblock(b, qb, extra=None):
                # PV+dn accumulator: slot j holds heads {j, j+? } ->
                # head h: partitions [64*(h%2), +33), bank h//2.
                pvp = ps.tile([128, 2, QB], F32, tag="pv", name=f"pv_{b}_{qb}")
                s_tiles = {0: emit_S(b, qb, 0), 1: emit_S(b, qb, 1)}
                flush_epi()   # previous q-block's tail, behind our first S
                for kt in range(KT):
                    if extra is not None:
                        extra(kt)
                    if kt + 2 < KT:
                        s_tiles[kt + 2] = emit_S(b, qb, kt + 2)
                    tA, tB = s_tiles.pop(kt)
                    eA = work.tile([128, 2, QB], BF16, tag="expt", bufs=8,
                                   name=f"eA_{b}_{qb}_{kt}")
                    eB = work.tile([128, 2, QB], BF16, tag="expt", bufs=8,
                                   name=f"eB_{b}_{qb}_{kt}")
                    nc.scalar.activation(eA[:], tA[:], AF.Exp, scale=SCALE)
                    nc.vector.tensor_scalar(
                        out=eB[:].bitcast(I16), in0=tB[:],
                        scalar1=float(SCH_A), scalar2=float(SCH_B),
                        op0=ALU.mult, op1=ALU.add)
                    for h in range(4):
                        rhs = (eA if h < 2 else eB)[:, h % 2, :]
                        p0 = 64 * (h % 2)
                        nc.tensor.matmul(
                            pvp[p0:p0 + DH + 1, h // 2, :],
                            V_sb[:, b * KT + kt, h, :],
                            rhs, start=(kt == 0), stop=(kt == KT - 1),
                            tile_position=(0, p0))
                # drain PSUM fast: dn rows via ACT (one op per partition
                # row covering both slots), att rows split ACT/DVE
                attf = work.tile([128, 2, QB], BF16, tag="attf", bufs=2,
                                 name=f"attf_{b}_{qb}")
                nc.scalar.copy(dn_st[32:33, :, :], pvp[32:33, :, :])
                nc.scalar.copy(dn_st[96:97, :, :], pvp[96:97, :, :])
                nc.vector.tensor_copy(attf[0:32, 0, :], pvp[0:32, 0, :])
                nc.vector.tensor_copy(attf[64:96, 0, :], pvp[64:96, 0, :])
                nc.scalar.copy(attf[0:32, 1, :], pvp[0:32, 1, :])
                nc.scalar.copy(attf[64:96, 1, :], pvp[64:96, 1, :])

                def epilogue():
                    ck = 2 * b + qb // 2
                    j0 = 4 * (qb % 2)
                    rec = work.tile([128, 2, QB], F32, tag="rec", bufs=2,
                                    name=f"rec_{b}_{qb}")
                    nc.vector.reciprocal_approx_fast(out=rec[:],
                                                     in_=dn_st[:])
                    rec_bf = work.tile([128, 2, QB], BF16, tag="recbf",
                                       bufs=2, name=f"recbf_{b}_{qb}")
                    nc.scalar.copy(rec_bf[:], rec[:])
                    recb = ps.tile([128, 2, QB], F32, tag="sp", bufs=3,
                                   name=f"recb_{b}_{qb}")
                    for j in range(2):
                        nc.tensor.matmul(recb[:, j, :], eb2s[:],
                                         rec_bf[:, j, :],
                                         start=True, stop=True)
                    recb_sb = work.tile([128, 2, QB], BF16, tag="recs",
                                        bufs=2, name=f"recs_{b}_{qb}")
                    nc.scalar.copy(recb_sb[:], recb[:])
                    att = work.tile([128, 2, QB], BF16, tag="att", bufs=2,
                                    name=f"att_{b}_{qb}")
                    for j in range(2):
                        for g in range(2):
                            sl = slice(64 * g, 64 * g + 32)
                            nc.gpsimd.tensor_tensor(
                                att[sl, j, :], attf[sl, j, :],
                                recb_sb[sl, j, :], ALU.mult)
                    # gather DMA: partitions {0-31, 64-95} -> block rows
                    # [64j+32g, +32) of each of the 4 dram blocks
                    for j in range(2):
                        for g in range(2):
                            r0 = 64 * j + 32 * g
                            nc.sync.dma_start(
                                a2a_in[ck][j0:j0 + 4, r0:r0 + 32, :]
                                .rearrange("j p q -> p j q"),
                                att[64 * g:64 * g + 32, j, :]
                                .rearrange("p (j q) -> p j q", j=4))

                pending_epi[0] = epilogue

            def a2a_exchange(ck):
                nc.gpsimd.collective_compute(
                    "AllToAll", ALU.bypass,
                    replica_groups=[list(range(N_CORES))],
                    ins=[a2a_in[ck].opt()], outs=[a2a_out[ck].opt()])

            # ---------------- phase C: out-proj + LN (per 128 rows) -------
            def outproj_load(ck):
                ab = work.tile([128, 8, 128], BF16, tag="a2asb", bufs=2,
                               name=f"ab_{ck}")
                for i in range(N_CORES):
                    nc.sync.dma_start(ab[:, i, :], a2a_out[ck][i])
                osb = work.tile([128, DIM], F32, tag="osb", bufs=2,
                                name=f"osb_{ck}")
                return ab, osb

            def outproj_mm(ck, ab, osb, nb):
                op = ps.tile([128, 512], F32, tag="sp", bufs=3,
                             name=f"op_{ck}_{nb}")
                for i in range(N_CORES):
                    nc.tensor.matmul(
                        op[:], ab[:, i, :],
                        wout_bf[:, i, nb * 512:(nb + 1) * 512],
                        start=(i == 0), stop=(i == N_CORES - 1))
                nc.vector.tensor_tensor(
                    osb[:, nb * 512:(nb + 1) * 512], op[:],
                    bout_bc[:, nb * 512:(nb + 1) * 512], ALU.add)

            def outproj_chunk(ck):
                ab, osb = outproj_load(ck)
                for nb in range(2):
                    outproj_mm(ck, ab, osb, nb)
                outproj_ln(ck, osb)

            def outproj_ln(ck, osb):
                # LayerNorm over the 1024 free dim
                stats = work.tile([128, 2, 6], F32, tag="stats", bufs=2,
                                  name=f"stats_{ck}")
                for sg in range(2):
                    nc.vector.bn_stats(out=stats[:, sg, :],
                                       in_=osb[:, sg * 512:(sg + 1) * 512])
                mv = work.tile([128, 2], F32, tag="mv", bufs=2,
                               name=f"mv_{ck}")
                nc.vector.bn_aggr(out=mv[:], in_=stats[:])
                # rstd = exp(-0.5 * ln(var + eps)) — stays in the exp/ln set
                lnv = work.tile([128, 1], F32, tag="lnv", bufs=2,
                                name=f"lnv_{ck}")
                nc.scalar.activation(out=lnv[:], in_=mv[:, 1:2], func=AF.Ln,
                                     bias=eps_sb[:], scale=1.0)
                rstd = work.tile([128, 1], F32, tag="rstd", bufs=2,
                                 name=f"rstd_{ck}")
                nc.scalar.activation(out=rstd[:], in_=lnv[:], func=AF.Exp,
                                     scale=-0.5)
                nc.vector.tensor_scalar(
                    out=osb[:], in0=osb[:], scalar1=mv[:, 0:1],
                    scalar2=rstd[:], op0=ALU.subtract, op1=ALU.mult)
                nc.gpsimd.tensor_tensor(osb[:], osb[:], gamma_bc[:], ALU.mult)
                nc.gpsimd.tensor_tensor(osb[:], osb[:], beta_bc[:], ALU.add)
                nc.sync.dma_start(out_d[ck * 128:(ck + 1) * 128, :], osb[:])

            # ---------------- schedule ----------------
            # per-kt emission hooks: stream the remaining projections and
            # weight loads into the attention pipeline instead of bursts
            def extra_00(kt):     # proj chunks 1..3 (rest of batch 0)
                if kt in (0, 4, 8):
                    proj_rowchunk(1 + kt // 4)

            attention_qblock(0, 0, extra_00)
            load_ln_consts()
            proj_rowchunk(4)
            attention_qblock(0, 1)
            flush_epi()
            a2a_exchange(0)
            proj_rowchunk(5)
            load_wout(0)
            load_wout(1)
            attention_qblock(0, 2)
            proj_rowchunk(6)
            load_wout(2)
            load_wout(3)
            attention_qblock(0, 3)
            flush_epi()
            a2a_exchange(1)
            proj_rowchunk(7)
            load_wout(4)
            load_wout(5)
            attention_qblock(1, 0)
            load_wout(6)
            load_wout(7)
            attention_qblock(1, 1)
            flush_epi()
            a2a_exchange(2)

            op_state = {}

            def extra_op(ck, k0=2):
                def hook(kt):
                    if kt == k0:
                        op_state[ck] = outproj_load(ck)
                    elif kt == k0 + 4:
                        outproj_mm(ck, *op_state[ck], 0)
                    elif kt == k0 + 8:
                        outproj_mm(ck, *op_state[ck], 1)
                    elif kt == k0 + 12:
                        outproj_ln(ck, op_state[ck][1])
                return hook

            def extra_op2(h1, h2):
                def hook(kt):
                    h1(kt)
                    h2(kt)
                return hook

            attention_qblock(1, 2, extra_op(0))
            attention_qblock(1, 3, extra_op2(extra_op(1, 1), extra_op(2, 3)))
            flush_epi()
            a2a_exchange(3)
            outproj_chunk(3)

    nc.compile()
    return nc


class _Runner:
    """Compile once; run the SPMD kernel on 8 cores via PJRT repeatedly."""

    def __init__(self):
        self.nc = _build()
        import jax
        from jax.sharding import Mesh, PartitionSpec, NamedSharding
        from jax.experimental.shard_map import shard_map
        from concourse import bass2jax
        bass2jax.install_neuronx_cc_hook()

        nc = self.nc
        part_name = (nc.partition_id_tensor.name
                     if nc.partition_id_tensor else None)
        in_names, out_names, out_avals = [], [], []
        for alloc in nc.m.functions[0].allocations:
            if not isinstance(alloc, mybir.MemoryLocationSet):
                continue
            name = alloc.memorylocations[0].name
            if alloc.kind == "ExternalInput":
                if name != part_name:
                    in_names.append(name)
            elif alloc.kind == "ExternalOutput":
                out_names.append(name)
                out_avals.append(jax.core.ShapedArray(
                    tuple(alloc.tensor_shape), mybir.dt.np(alloc.dtype)))
        self.in_names = list(in_names)
        self.out_names = out_names
        self.out_avals = out_avals
        all_in_names = in_names + out_names
        if part_name is not None:
            all_in_names = all_in_names + [part_name]

        def _body(*args):
            operands = list(args)
            if part_name is not None:
                operands.append(bass2jax.partition_id_tensor())
            outs = bass2jax._bass_exec_p.bind(
                *operands, out_avals=tuple(out_avals),
                in_names=tuple(all_in_names), out_names=tuple(out_names),
                lowering_input_output_aliases=(),
                sim_require_finite=True, sim_require_nnan=True, nc=nc)
            return tuple(outs)

        devices = jax.devices()[:N_CORES]
        mesh = Mesh(np.asarray(devices), ("core",))
        self.sharding = NamedSharding(mesh, PartitionSpec("core"))
        nin = len(self.in_names) + len(out_names)
        self.fn = jax.jit(shard_map(
            _body, mesh=mesh, in_specs=(PartitionSpec("core"),) * nin,
            out_specs=(PartitionSpec("core"),) * len(out_names),
            check_rep=False))
        self.jax = jax

    def stage(self, in_maps):
        """Concatenate per-core inputs + zero outputs; device_put with the
        mesh sharding so steady-state calls skip any resharding."""
        concat = [np.concatenate([m[name] for m in in_maps], axis=0)
                  for name in self.in_names]
        zeros = [np.zeros((N_CORES * a.shape[0], *a.shape[1:]), a.dtype)
                 for a in self.out_avals]
        return [self.jax.device_put(x, self.sharding) for x in concat + zeros]

    def run_staged(self, staged):
        outs = self.fn(*staged)
        self.jax.block_until_ready(outs)
        return outs

    def run(self, in_maps):
        outs = self.run_staged(self.stage(in_maps))
        return [
            {name: np.asarray(outs[i]).reshape(
                N_CORES, *self.out_avals[i].shape)[c]
             for i, name in enumerate(self.out_names)}
            for c in range(N_CORES)
        ]


_RUNNER = None


def _get_runner():
    global _RUNNER
    if _RUNNER is None:
        _RUNNER = _Runner()
    return _RUNNER


def _make_in_maps(x, w_qkv, w_out, b_out, ln_gamma, ln_beta):
    x = np.asarray(x, dtype=np.float32)
    w_qkv = np.asarray(w_qkv, dtype=np.float32)
    w_out = np.asarray(w_out, dtype=np.float32)
    b_out = np.asarray(b_out, dtype=np.float32)
    ln_gamma = np.asarray(ln_gamma, dtype=np.float32)
    ln_beta = np.asarray(ln_beta, dtype=np.float32)

    xT = np.ascontiguousarray(x.reshape(ROWS, DIM).T)
    in_maps = []
    for c in range(N_CORES):
        h0 = HPC * c * DH
        cols = np.concatenate([
            w_qkv[:, h0:h0 + HPC * DH],
            w_qkv[:, DIM + h0:DIM + h0 + HPC * DH],
            w_qkv[:, 2 * DIM + h0:2 * DIM + h0 + HPC * DH],
        ], axis=1)
        in_maps.append({
            "xT": xT,
            "wqkv": np.ascontiguousarray(cols),
            "wout": w_out,
            "bout": b_out,
            "gamma": ln_gamma,
            "beta": ln_beta,
        })
    return in_maps


def kernel(x, w_qkv, w_out, b_out, ln_gamma, ln_beta):
    runner = _get_runner()
    in_maps = _make_in_maps(x, w_qkv, w_out, b_out, ln_gamma, ln_beta)
    results = runner.run(in_maps)
    # per-core out rows: [chunk(4), 128]; global row = 1024*ck + 128*c + r
    full = np.empty((ROWS, DIM), dtype=np.float32)
    for c in range(N_CORES):
        o = results[c]["out"]
        for ck in range(NCK):
            r0 = 1024 * ck + 128 * c
            full[r0:r0 + 128] = o[ck * 128:(ck + 1) * 128]
    return full.reshape(B, N, DIM)


# revision 24
# speedup vs baseline: 1.1422x; 1.0664x over previous
"""Fused multi-head attention block (QKV proj + softmax attention + out-proj
+ LayerNorm) for Trainium2, sharded over 8 NeuronCores.

Sharding: tensor-parallel over heads. Core c owns heads [4c, 4c+4).

v2 design (vs the 508us baseline):
  - Denominator matmuls eliminated: PV lhsT is [V_h | ones] (M=33), so the
    per-head softmax denominator accumulates on PSUM partition 32/96 of the
    same stream that computes P@V.  Saves ~1/3 of attention PE cycles.
  - exp split across two engines so the Scalar engine stops pacing the PE
    (which kept HAM-throttling to half clock): ACT does exact exp for heads
    0-1 (eA); DVE does a Schraudolph bf16 exp (t = S*a+b -> int16 ->
    bitcast bf16, ~3% rel err, row-common part cancels in softmax) for
    heads 2-3 (eB).
  - Epilogue restructured: PSUM drained fast (ACT copies dn rows, DVE
    copies att rows), normalization (reciprocal + eb2 broadcast matmul +
    multiply) deferred; the final multiply runs on GPSIMD (SBUF-only).
  - Collectives issued from the Sync queue; xt DMAs alternate
    sync/gpsimd; startup DMAs (wqkv + first x chunk) split across 5 engine
    queues so the first matmul starts at ~16us instead of ~43us.

dtypes: fp32 storage; QKV and S^T matmuls run as float32r; exp(S^T), PV
and out-proj use bf16.  LayerNorm rstd uses exp(-0.5*ln(var+eps)) so the
Scalar engine keeps a single activation table set loaded.
"""
import sys

for _p in ("/opt/trn_rl_repo", "/root/.axon_site/_ro/trn_rl_repo"):
    if _p not in sys.path:
        sys.path.insert(0, _p)

import numpy as np

import concourse.bass as bass
import concourse.tile as tile
from concourse import bacc, mybir
from concourse.masks import make_identity

F32 = mybir.dt.float32
F32R = mybir.dt.float32r
BF16 = mybir.dt.bfloat16
I16 = mybir.dt.int16
I32 = mybir.dt.int32
AF = mybir.ActivationFunctionType
ALU = mybir.AluOpType

N_CORES = 8
B, N, DIM = 2, 2048, 1024
HEADS, DH = 32, 32           # 32 heads x 32 dim/head
HPC = HEADS // N_CORES       # 4 heads per core
ROWS = B * N                 # 4096 global rows
SCALE = DH ** -0.5
EPS = 1e-6
KT = N // 128                # 16 key tiles per batch
QB = 512                     # q-block width
NQB = N // QB                # 4 q-blocks per batch
RC = 512                     # projection row-chunk
NRC = ROWS // RC             # 16 row chunks
NCK = 4                      # a2a chunks; chunk k = global rows [1024k, 1024k+1024)
                             # core c owns rows 1024k + 128c .. +128 of each chunk

# Schraudolph exp in bf16: exp(x) ~ bitcast_bf16(int16(x * A + B))
# (hardware rounds to nearest on the f32->int16 convert).
SCH_A = (2.0 ** 7) / np.log(2.0) * SCALE   # folds the 1/sqrt(dh) scale in
SCH_B = 127.0 * 2 ** 7 - 4.5


def _build():
    nc = bacc.Bacc("TRN2", target_bir_lowering=False, debug=False,
                   num_devices=N_CORES)

    xT_d = nc.dram_tensor("xT", [DIM, ROWS], BF16, kind="ExternalInput").ap()
    wqkv_d = nc.dram_tensor("wqkv", [DIM, 3 * HPC * DH], BF16,
                            kind="ExternalInput").ap()
    wout_d = nc.dram_tensor("wout", [DIM, DIM], BF16,
                            kind="ExternalInput").ap()
    bout_d = nc.dram_tensor("bout", [DIM], F32, kind="ExternalInput").ap()
    gamma_d = nc.dram_tensor("gamma", [DIM], F32, kind="ExternalInput").ap()
    beta_d = nc.dram_tensor("beta", [DIM], F32, kind="ExternalInput").ap()
    # rows: [chunk(4), 128]; global row = 1024*ck + 128*core + r
    out_d = nc.dram_tensor("out", [NCK * 128, DIM], F32,
                           kind="ExternalOutput").ap()

    with tile.TileContext(nc) as tc:
        with (
            tc.tile_pool(name="const", bufs=1) as const,
            tc.tile_pool(name="work", bufs=1) as work,
            tc.tile_pool(name="ps", bufs=1, space="PSUM") as ps,
            tc.tile_pool(name="dram", bufs=1, space="DRAM") as dram,
        ):
            # ---------------- constants / weights ----------------
            # wqkv split across two queues to shorten the startup window
            wqkv_sb = const.tile([128, 8, 3 * HPC * DH], BF16)
            wqkv_r = wqkv_d.rearrange("(kc p) m -> p kc m", p=128)
            for kc in range(4):
                nc.scalar.dma_start(wqkv_sb[:, 2 * kc:2 * kc + 2, :],
                                    wqkv_r[:, 2 * kc:2 * kc + 2, :])
            ident = const.tile([128, 128], BF16)
            make_identity(nc, ident[:])
            eps_sb = const.tile([128, 1], F32)
            nc.vector.memset(eps_sb[:], EPS)
            # slot-broadcast matrix: contracting eb2s against a [128, q]
            # tile whose row 32 / 96 holds the per-head reciprocal
            # denominator broadcasts row 32 -> partitions [0,32) and row
            # 96 -> partitions [64,96); all other output rows get 0.
            eb2s = const.tile([128, 128], BF16)
            nc.vector.memset(eb2s[:], 0.0)
            nc.vector.memset(eb2s[32:33, 0:32], 1.0)
            nc.vector.memset(eb2s[96:97, 64:96], 1.0)
            # denominator staging rows, both slots in one tile (junk rows
            # stay 1.0 so the full-tile fast reciprocal stays finite)
            dn_st = const.tile([128, 2, QB], F32)
            nc.vector.memset(dn_st[:], 1.0)
            # warm the ACT exp table (the only table the kernel uses)
            scr = const.tile([128, 1], F32)
            nc.scalar.activation(out=scr[:], in_=eps_sb[:], func=AF.Exp,
                                 scale=1.0)
            # row-broadcast vectors [128, 1024]; DMAs deferred past the
            # startup window so they don't queue ahead of the xT loads
            bout_bc = const.tile([128, DIM], F32)
            gamma_bc = const.tile([128, DIM], F32)
            beta_bc = const.tile([128, DIM], F32)

            def load_ln_consts():
                for bc, src_d in ((bout_bc, bout_d), (gamma_bc, gamma_d),
                                  (beta_bc, beta_d)):
                    nc.sync.dma_start(out=bc[:], in_=bass.AP(
                        tensor=src_d.tensor, offset=src_d.offset,
                        ap=[[0, 128], [1, DIM]]))
            # w_out -> bf16 [128, 8, 1024]
            wout_bf = const.tile([128, 8, DIM], BF16)

            # ---------------- persistent activations ----------------
            qT_sb = const.tile([128, ROWS], F32R)   # 4h x 32d on partitions
            kT_sb = const.tile([128, ROWS], F32R)
            # V with a ones column per head: [key%128, ktile, head, 33]
            V_sb = const.tile([128, 2 * KT, HPC, DH + 1], BF16)
            nc.vector.memset(V_sb[:, :, :, DH:DH + 1], 1.0)

            # ---------------- dram bounce buffers ----------------
            a2a_in = [dram.tile([N_CORES, 128, 128], BF16, name=f"a2ai_{k}")
                      for k in range(NCK)]
            a2a_out = [dram.tile([N_CORES, 128, 128], BF16, name=f"a2ao_{k}")
                       for k in range(NCK)]

            # ---------------- phase A: projections ----------------
            def proj_rowchunk(rc, split=False):
                xt = work.tile([128, 8, RC], BF16, tag="xt", bufs=3,
                               name=f"xt_{rc}")
                src = (xT_d[:, rc * RC:(rc + 1) * RC]
                       .rearrange("(kc p) n -> p kc n", p=128))
                if split:
                    # startup: split into 4 DMAs (parallel hw queues) on
                    # sync + 2 on gpsimd
                    for kc in range(3):
                        nc.sync.dma_start(xt[:, 2 * kc:2 * kc + 2, :],
                                          src[:, 2 * kc:2 * kc + 2, :])
                    nc.gpsimd.dma_start(xt[:, 6:8, :], src[:, 6:8, :])
                else:
                    # keep the gpsimd queue free for collectives: even
                    # chunks on sync, odd chunks on the scalar hwdge
                    dma_eng = nc.sync if rc % 2 == 0 else nc.scalar
                    dma_eng.dma_start(xt[:], src)
                for name, mofs, dst in (("q", 0, qT_sb), ("k", 128, kT_sb)):
                    pp = ps.tile([128, RC], F32, tag="sp", bufs=3,
                                 name=f"pp_{name}_{rc}")
                    for kc in range(8):
                        nc.tensor.matmul(
                            pp[:], wqkv_sb[:, kc, mofs:mofs + 128],
                            xt[:, kc, :], start=(kc == 0), stop=(kc == 7))
                    nc.vector.tensor_copy(dst[:, rc * RC:(rc + 1) * RC], pp[:])
                # v: project (vT layout), cast bf16, DMA-transpose into V_sb
                pv_ = ps.tile([128, RC], F32, tag="sp", bufs=3,
                               name=f"pp_v_{rc}")
                for kc in range(8):
                    nc.tensor.matmul(
                        pv_[:], wqkv_sb[:, kc, 256:384], xt[:, kc, :],
                        start=(kc == 0), stop=(kc == 7))
                vt = work.tile([128, RC], BF16, tag="vt", bufs=2,
                               name=f"vt_{rc}")
                nc.vector.tensor_copy(vt[:], pv_[:])
                for i in range(RC // 128):
                    tp = ps.tile([128, 128], BF16, tag="sp", bufs=3,
                                 name=f"tp_{rc}_{i}")
                    nc.tensor.matmul(
                        tp[:], vt[:, i * 128:(i + 1) * 128], ident[:],
                        is_transpose=True, start=True, stop=True)
                    kt_abs = rc * (RC // 128) + i
                    nc.vector.tensor_copy(
                        V_sb[:, kt_abs, :, 0:DH],
                        tp[:].rearrange("p (h d) -> p h d", h=HPC))

            def load_wout(j):
                nc.sync.dma_start(wout_bf[:, j, :],
                                  wout_d[j * 128:(j + 1) * 128, :])

            proj_rowchunk(0, split=True)    # enough rows to start qb0

            # ---------------- phase B: attention ----------------
            # Software-pipelined per q-block: S two key-tiles ahead; ACT
            # exp paces heads 0-1, DVE Schraudolph paces heads 2-3; PV
            # (with fused denominator column) trails by one tile.
            def emit_S(b, qb, kt):
                q0 = b * N + qb * QB
                k0 = b * N + kt * 128
                tA = ps.tile([128, 2, QB], F32, tag="sp", bufs=3,
                             name=f"sA_{b}_{qb}_{kt}")
                tB = ps.tile([128, 2, QB], F32, tag="sp", bufs=3,
                             name=f"sB_{b}_{qb}_{kt}")
                for h in range(4):
                    t = tA if h < 2 else tB
                    nc.tensor.matmul(
                        t[:, h % 2, :],
                        kT_sb[32 * h:32 * h + 32, k0:k0 + 128],
                        qT_sb[32 * h:32 * h + 32, q0:q0 + QB],
                        start=True, stop=True, tile_position=(32 * h, 0))
                return tA, tB

            pending_epi = [None]

            def flush_epi():
                if pending_epi[0] is not None:
                    fn = pending_epi[0]
                    pending_epi[0] = None
                    fn()

            def attention_qblock(b, qb, extra=None):
                # PV+dn accumulator: slot j holds heads {j, j+? } ->
                # head h: partitions [64*(h%2), +33), bank h//2.
                pvp = ps.tile([128, 2, QB], F32, tag="pv", name=f"pv_{b}_{qb}")
                s_tiles = {0: emit_S(b, qb, 0), 1: emit_S(b, qb, 1)}
                flush_epi()   # previous q-block's tail, behind our first S
                for kt in range(KT):
                    if extra is not None:
                        extra(kt)
                    if kt + 2 < KT:
                        s_tiles[kt + 2] = emit_S(b, qb, kt + 2)
                    tA, tB = s_tiles.pop(kt)
                    eA = work.tile([128, 2, QB], BF16, tag="expt", bufs=8,
                                   name=f"eA_{b}_{qb}_{kt}")
                    eB = work.tile([128, 2, QB], BF16, tag="expt", bufs=8,
                                   name=f"eB_{b}_{qb}_{kt}")
                    nc.scalar.activation(eA[:], tA[:], AF.Exp, scale=SCALE)
                    nc.vector.tensor_scalar(
                        out=eB[:].bitcast(I16), in0=tB[:],
                        scalar1=float(SCH_A), scalar2=float(SCH_B),
                        op0=ALU.mult, op1=ALU.add)
                    for h in range(4):
                        rhs = (eA if h < 2 else eB)[:, h % 2, :]
                        p0 = 64 * (h % 2)
                        nc.tensor.matmul(
                            pvp[p0:p0 + DH + 1, h // 2, :],
                            V_sb[:, b * KT + kt, h, :],
                            rhs, start=(kt == 0), stop=(kt == KT - 1),
                            tile_position=(0, p0))
                # drain PSUM fast: dn rows via ACT (one op per partition
                # row covering both slots), att rows split ACT/DVE
                attf = work.tile([128, 2, QB], BF16, tag="attf", bufs=2,
                                 name=f"attf_{b}_{qb}")
                nc.scalar.copy(dn_st[32:33, :, :], pvp[32:33, :, :])
                nc.scalar.copy(dn_st[96:97, :, :], pvp[96:97, :, :])
                nc.vector.tensor_copy(attf[0:32, 0, :], pvp[0:32, 0, :])
                nc.vector.tensor_copy(attf[64:96, 0, :], pvp[64:96, 0, :])
                nc.scalar.copy(attf[0:32, 1, :], pvp[0:32, 1, :])
                nc.scalar.copy(attf[64:96, 1, :], pvp[64:96, 1, :])

                def epilogue():
                    ck = 2 * b + qb // 2
                    j0 = 4 * (qb % 2)
                    rec = work.tile([128, 2, QB], F32, tag="rec", bufs=2,
                                    name=f"rec_{b}_{qb}")
                    nc.vector.reciprocal_approx_fast(out=rec[:],
                                                     in_=dn_st[:])
                    rec_bf = work.tile([128, 2, QB], BF16, tag="recbf",
                                       bufs=2, name=f"recbf_{b}_{qb}")
                    nc.scalar.copy(rec_bf[:], rec[:])
                    recb = ps.tile([128, 2, QB], F32, tag="sp", bufs=3,
                                   name=f"recb_{b}_{qb}")
                    for j in range(2):
                        nc.tensor.matmul(recb[:, j, :], eb2s[:],
                                         rec_bf[:, j, :],
                                         start=True, stop=True)
                    recb_sb = work.tile([128, 2, QB], BF16, tag="recs",
                                        bufs=2, name=f"recs_{b}_{qb}")
                    nc.scalar.copy(recb_sb[:], recb[:])
                    att = work.tile([128, 2, QB], BF16, tag="att", bufs=2,
                                    name=f"att_{b}_{qb}")
                    for j in range(2):
                        for g in range(2):
                            sl = slice(64 * g, 64 * g + 32)
                            nc.gpsimd.tensor_tensor(
                                att[sl, j, :], attf[sl, j, :],
                                recb_sb[sl, j, :], ALU.mult)
                    # gather DMA: partitions {0-31, 64-95} -> block rows
                    # [64j+32g, +32) of each of the 4 dram blocks
                    for j in range(2):
                        for g in range(2):
                            r0 = 64 * j + 32 * g
                            nc.sync.dma_start(
                                a2a_in[ck][j0:j0 + 4, r0:r0 + 32, :]
                                .rearrange("j p q -> p j q"),
                                att[64 * g:64 * g + 32, j, :]
                                .rearrange("p (j q) -> p j q", j=4))

                pending_epi[0] = epilogue

            def a2a_exchange(ck):
                nc.gpsimd.collective_compute(
                    "AllToAll", ALU.bypass,
                    replica_groups=[list(range(N_CORES))],
                    ins=[a2a_in[ck].opt()], outs=[a2a_out[ck].opt()])

            # ---------------- phase C: out-proj + LN (per 128 rows) -------
            def outproj_load(ck):
                ab = work.tile([128, 8, 128], BF16, tag="a2asb", bufs=2,
                               name=f"ab_{ck}")
                for i in range(N_CORES):
                    nc.sync.dma_start(ab[:, i, :], a2a_out[ck][i])
                osb = work.tile([128, DIM], F32, tag="osb", bufs=2,
                                name=f"osb_{ck}")
                return ab, osb

            def outproj_mm(ck, ab, osb, nb):
                op = ps.tile([128, 512], F32, tag="sp", bufs=3,
                             name=f"op_{ck}_{nb}")
                for i in range(N_CORES):
                    nc.tensor.matmul(
                        op[:], ab[:, i, :],
                        wout_bf[:, i, nb * 512:(nb + 1) * 512],
                        start=(i == 0), stop=(i == N_CORES - 1))
                nc.vector.tensor_tensor(
                    osb[:, nb * 512:(nb + 1) * 512], op[:],
                    bout_bc[:, nb * 512:(nb + 1) * 512], ALU.add)

            def outproj_chunk(ck, tail=False):
                ab, osb = outproj_load(ck)
                for nb in range(2):
                    outproj_mm(ck, ab, osb, nb)
                outproj_ln(ck, osb, tail=tail)

            def outproj_ln(ck, osb, tail=False):
                # LayerNorm over the 1024 free dim
                stats = work.tile([128, 2, 6], F32, tag="stats", bufs=2,
                                  name=f"stats_{ck}")
                for sg in range(2):
                    nc.vector.bn_stats(out=stats[:, sg, :],
                                       in_=osb[:, sg * 512:(sg + 1) * 512])
                mv = work.tile([128, 2], F32, tag="mv", bufs=2,
                               name=f"mv_{ck}")
                nc.vector.bn_aggr(out=mv[:], in_=stats[:])
                # rstd = 1/sqrt(var + eps), DVE-only: float quake seed
                # (bits(y) = C - bits(x)/2 done in fp32) + 2 Newton steps.
                # Keeps the Scalar engine free of Ln (no ACT table swaps).
                lw = work.tile([128, 4], F32, tag="lnw", bufs=2,
                               name=f"lnw_{ck}")
                veps = lw[:, 0:1]
                nc.vector.tensor_scalar(out=veps, in0=mv[:, 1:2],
                                        scalar1=EPS, scalar2=None,
                                        op0=ALU.add)
                nc.vector.tensor_copy(lw[:, 1:2], veps.bitcast(I32))
                rstd = work.tile([128, 1], F32, tag="rstd", bufs=2,
                                 name=f"rstd_{ck}")
                nc.vector.tensor_scalar(
                    out=rstd[:].bitcast(I32), in0=lw[:, 1:2],
                    scalar1=-0.5, scalar2=1597463007.0,
                    op0=ALU.mult, op1=ALU.add)
                for _ in range(2):
                    nc.vector.tensor_tensor(lw[:, 2:3], rstd[:], rstd[:],
                                            ALU.mult)
                    nc.vector.tensor_tensor(lw[:, 2:3], lw[:, 2:3], veps,
                                            ALU.mult)
                    nc.vector.tensor_scalar(
                        out=lw[:, 2:3], in0=lw[:, 2:3], scalar1=-0.5,
                        scalar2=1.5, op0=ALU.mult, op1=ALU.add)
                    nc.vector.tensor_tensor(rstd[:], rstd[:], lw[:, 2:3],
                                            ALU.mult)
                nc.vector.tensor_scalar(
                    out=osb[:], in0=osb[:], scalar1=mv[:, 0:1],
                    scalar2=rstd[:], op0=ALU.subtract, op1=ALU.mult)
                eng = nc.vector if tail else nc.gpsimd
                eng.tensor_tensor(osb[:], osb[:], gamma_bc[:], ALU.mult)
                eng.tensor_tensor(osb[:], osb[:], beta_bc[:], ALU.add)
                nc.sync.dma_start(out_d[ck * 128:(ck + 1) * 128, :], osb[:])

            # ---------------- schedule ----------------
            # per-kt emission hooks: stream the remaining projections and
            # weight loads into the attention pipeline instead of bursts
            def extra_00(kt):     # proj chunks 1..3 (rest of batch 0)
                if kt in (0, 4, 8):
                    proj_rowchunk(1 + kt // 4)

            attention_qblock(0, 0, extra_00)
            load_ln_consts()
            proj_rowchunk(4)
            attention_qblock(0, 1)
            flush_epi()
            a2a_exchange(0)
            proj_rowchunk(5)
            load_wout(0)
            load_wout(1)
            attention_qblock(0, 2)
            proj_rowchunk(6)
            load_wout(2)
            load_wout(3)
            attention_qblock(0, 3)
            flush_epi()
            a2a_exchange(1)
            proj_rowchunk(7)
            load_wout(4)
            load_wout(5)
            attention_qblock(1, 0)
            load_wout(6)
            load_wout(7)
            attention_qblock(1, 1)
            flush_epi()
            a2a_exchange(2)

            op_state = {}

            def extra_op(ck, k0=2):
                def hook(kt):
                    if kt == k0:
                        op_state[ck] = outproj_load(ck)
                    elif kt == k0 + 4:
                        outproj_mm(ck, *op_state[ck], 0)
                    elif kt == k0 + 8:
                        outproj_mm(ck, *op_state[ck], 1)
                    elif kt == k0 + 12:
                        outproj_ln(ck, op_state[ck][1])
                return hook

            def extra_op2(h1, h2):
                def hook(kt):
                    h1(kt)
                    h2(kt)
                return hook

            attention_qblock(1, 2, extra_op(0))
            attention_qblock(1, 3, extra_op2(extra_op(1, 1), extra_op(2, 3)))
            flush_epi()
            a2a_exchange(3)
            outproj_chunk(3, tail=True)

    nc.compile()
    return nc


class _Runner:
    """Compile once; run the SPMD kernel on 8 cores via PJRT repeatedly."""

    def __init__(self):
        self.nc = _build()
        import jax
        from jax.sharding import Mesh, PartitionSpec, NamedSharding
        from jax.experimental.shard_map import shard_map
        from concourse import bass2jax
        bass2jax.install_neuronx_cc_hook()

        nc = self.nc
        part_name = (nc.partition_id_tensor.name
                     if nc.partition_id_tensor else None)
        in_names, out_names, out_avals = [], [], []
        for alloc in nc.m.functions[0].allocations:
            if not isinstance(alloc, mybir.MemoryLocationSet):
                continue
            name = alloc.memorylocations[0].name
            if alloc.kind == "ExternalInput":
                if name != part_name:
                    in_names.append(name)
            elif alloc.kind == "ExternalOutput":
                out_names.append(name)
                out_avals.append(jax.core.ShapedArray(
                    tuple(alloc.tensor_shape), mybir.dt.np(alloc.dtype)))
        self.in_names = list(in_names)
        self.out_names = out_names
        self.out_avals = out_avals
        all_in_names = in_names + out_names
        if part_name is not None:
            all_in_names = all_in_names + [part_name]

        def _body(*args):
            operands = list(args)
            if part_name is not None:
                operands.append(bass2jax.partition_id_tensor())
            outs = bass2jax._bass_exec_p.bind(
                *operands, out_avals=tuple(out_avals),
                in_names=tuple(all_in_names), out_names=tuple(out_names),
                lowering_input_output_aliases=(),
                sim_require_finite=True, sim_require_nnan=True, nc=nc)
            return tuple(outs)

        devices = jax.devices()[:N_CORES]
        mesh = Mesh(np.asarray(devices), ("core",))
        self.sharding = NamedSharding(mesh, PartitionSpec("core"))
        nin = len(self.in_names) + len(out_names)
        self.fn = jax.jit(shard_map(
            _body, mesh=mesh, in_specs=(PartitionSpec("core"),) * nin,
            out_specs=(PartitionSpec("core"),) * len(out_names),
            check_rep=False))
        self.jax = jax

    def stage(self, in_maps):
        """Concatenate per-core inputs + zero outputs; device_put with the
        mesh sharding so steady-state calls skip any resharding."""
        concat = [np.concatenate([m[name] for m in in_maps], axis=0)
                  for name in self.in_names]
        zeros = [np.zeros((N_CORES * a.shape[0], *a.shape[1:]), a.dtype)
                 for a in self.out_avals]
        return [self.jax.device_put(x, self.sharding) for x in concat + zeros]

    def run_staged(self, staged):
        outs = self.fn(*staged)
        self.jax.block_until_ready(outs)
        return outs

    def run(self, in_maps):
        outs = self.run_staged(self.stage(in_maps))
        return [
            {name: np.asarray(outs[i]).reshape(
                N_CORES, *self.out_avals[i].shape)[c]
             for i, name in enumerate(self.out_names)}
            for c in range(N_CORES)
        ]


_RUNNER = None


def _get_runner():
    global _RUNNER
    if _RUNNER is None:
        _RUNNER = _Runner()
    return _RUNNER


def _make_in_maps(x, w_qkv, w_out, b_out, ln_gamma, ln_beta):
    bf16 = mybir.dt.np(BF16)
    x = np.asarray(x, dtype=np.float32)
    w_qkv = np.asarray(w_qkv, dtype=np.float32)
    w_out = np.asarray(w_out, dtype=np.float32).astype(bf16)
    b_out = np.asarray(b_out, dtype=np.float32)
    ln_gamma = np.asarray(ln_gamma, dtype=np.float32)
    ln_beta = np.asarray(ln_beta, dtype=np.float32)

    xT = np.ascontiguousarray(x.reshape(ROWS, DIM).T).astype(bf16)
    in_maps = []
    for c in range(N_CORES):
        h0 = HPC * c * DH
        cols = np.concatenate([
            w_qkv[:, h0:h0 + HPC * DH],
            w_qkv[:, DIM + h0:DIM + h0 + HPC * DH],
            w_qkv[:, 2 * DIM + h0:2 * DIM + h0 + HPC * DH],
        ], axis=1)
        in_maps.append({
            "xT": xT,
            "wqkv": np.ascontiguousarray(cols).astype(bf16),
            "wout": w_out,
            "bout": b_out,
            "gamma": ln_gamma,
            "beta": ln_beta,
        })
    return in_maps


def kernel(x, w_qkv, w_out, b_out, ln_gamma, ln_beta):
    runner = _get_runner()
    in_maps = _make_in_maps(x, w_qkv, w_out, b_out, ln_gamma, ln_beta)
    results = runner.run(in_maps)
    # per-core out rows: [chunk(4), 128]; global row = 1024*ck + 128*c + r
    full = np.empty((ROWS, DIM), dtype=np.float32)
    for c in range(N_CORES):
        o = results[c]["out"]
        for ck in range(NCK):
            r0 = 1024 * ck + 128 * c
            full[r0:r0 + 128] = o[ck * 128:(ck + 1) * 128]
    return full.reshape(B, N, DIM)


# revision 43
# speedup vs baseline: 1.2359x; 1.0821x over previous
"""Fused multi-head attention block (QKV proj + softmax attention + out-proj
+ LayerNorm) for Trainium2, sharded over 8 NeuronCores.

Sharding: tensor-parallel over heads. Core c owns heads [4c, 4c+4).

v2 design (vs the 508us baseline):
  - Denominator matmuls eliminated: PV lhsT is [V_h | ones] (M=33), so the
    per-head softmax denominator accumulates on PSUM partition 32/96 of the
    same stream that computes P@V.  Saves ~1/3 of attention PE cycles.
  - exp split across two engines so the Scalar engine stops pacing the PE
    (which kept HAM-throttling to half clock): ACT does exact exp for heads
    0-1 (eA); DVE does a Schraudolph bf16 exp (t = S*a+b -> int16 ->
    bitcast bf16, ~3% rel err, row-common part cancels in softmax) for
    heads 2-3 (eB).
  - Epilogue restructured: PSUM drained fast (ACT copies dn rows, DVE
    copies att rows), normalization (reciprocal + eb2 broadcast matmul +
    multiply) deferred; the final multiply runs on GPSIMD (SBUF-only).
  - Collectives issued from the Sync queue; xt DMAs alternate
    sync/gpsimd; startup DMAs (wqkv + first x chunk) split across 5 engine
    queues so the first matmul starts at ~16us instead of ~43us.

dtypes: fp32 storage; QKV and S^T matmuls run as float32r; exp(S^T), PV
and out-proj use bf16.  LayerNorm rstd uses exp(-0.5*ln(var+eps)) so the
Scalar engine keeps a single activation table set loaded.
"""
import sys

for _p in ("/opt/trn_rl_repo", "/root/.axon_site/_ro/trn_rl_repo"):
    if _p not in sys.path:
        sys.path.insert(0, _p)

import numpy as np

import concourse.bass as bass
import concourse.tile as tile
from concourse import bacc, mybir
from concourse.masks import make_identity

F32 = mybir.dt.float32
F32R = mybir.dt.float32r
BF16 = mybir.dt.bfloat16
I16 = mybir.dt.int16
I32 = mybir.dt.int32
AF = mybir.ActivationFunctionType
ALU = mybir.AluOpType

N_CORES = 8
B, N, DIM = 2, 2048, 1024
HEADS, DH = 32, 32           # 32 heads x 32 dim/head
HPC = HEADS // N_CORES       # 4 heads per core
ROWS = B * N                 # 4096 global rows
SCALE = DH ** -0.5
EPS = 1e-6
KT = N // 128                # 16 key tiles per batch
QB = 512                     # q-block width
NQB = N // QB                # 4 q-blocks per batch
RC = 512                     # projection row-chunk
NRC = ROWS // RC             # 16 row chunks
NCK = 4                      # a2a chunks; chunk k = global rows [1024k, 1024k+1024)
                             # core c owns rows 1024k + 128c .. +128 of each chunk

# Schraudolph exp in bf16: exp(x) ~ bitcast_bf16(int16(x * A + B))
# (hardware rounds to nearest on the f32->int16 convert).
SCH_A = (2.0 ** 7) / np.log(2.0) * SCALE   # folds the 1/sqrt(dh) scale in
SCH_B = 127.0 * 2 ** 7 - 4.5


def _build():
    nc = bacc.Bacc("TRN2", target_bir_lowering=False, debug=False,
                   num_devices=N_CORES)

    xT_d = nc.dram_tensor("xT", [DIM, ROWS], BF16, kind="ExternalInput").ap()
    wqkv_d = nc.dram_tensor("wqkv", [DIM, 3 * HPC * DH], BF16,
                            kind="ExternalInput").ap()
    wout_d = nc.dram_tensor("wout", [DIM, DIM], BF16,
                            kind="ExternalInput").ap()
    bout_d = nc.dram_tensor("bout", [DIM], F32, kind="ExternalInput").ap()
    gamma_d = nc.dram_tensor("gamma", [DIM], F32, kind="ExternalInput").ap()
    beta_d = nc.dram_tensor("beta", [DIM], F32, kind="ExternalInput").ap()
    # rows: [chunk(4), 128]; global row = 1024*ck + 128*core + r
    out_d = nc.dram_tensor("out", [NCK * 128, DIM], F32,
                           kind="ExternalOutput").ap()

    with tile.TileContext(nc) as tc:
        with (
            tc.tile_pool(name="const", bufs=1) as const,
            tc.tile_pool(name="work", bufs=1) as work,
            tc.tile_pool(name="ps", bufs=1, space="PSUM") as ps,
            tc.tile_pool(name="dram", bufs=1, space="DRAM") as dram,
        ):
            # ---------------- constants / weights ----------------
            # wqkv split across two queues to shorten the startup window
            wqkv_sb = const.tile([128, 8, 3 * HPC * DH], BF16)
            wqkv_r = wqkv_d.rearrange("(kc p) m -> p kc m", p=128)
            for kc in range(4):
                nc.scalar.dma_start(wqkv_sb[:, 2 * kc:2 * kc + 2, :],
                                    wqkv_r[:, 2 * kc:2 * kc + 2, :])
            ident = const.tile([128, 128], BF16)
            make_identity(nc, ident[:])
            eps_sb = const.tile([128, 1], F32)
            nc.vector.memset(eps_sb[:], EPS)
            # denominator staging rows, both slots in one tile (junk rows
            # stay 1.0 so the full-tile fast reciprocal stays finite)
            dn_st = const.tile([128, 2, QB], F32)
            nc.vector.memset(dn_st[:], 1.0)
            # warm the ACT exp table (the only table the kernel uses)
            scr = const.tile([128, 1], F32)
            nc.scalar.activation(out=scr[:], in_=eps_sb[:], func=AF.Exp,
                                 scale=1.0)
            # row-broadcast vectors [128, 1024]; DMAs deferred past the
            # startup window so they don't queue ahead of the xT loads
            bout_bc = const.tile([128, DIM], F32)
            gamma_bc = const.tile([128, DIM], F32)
            beta_bc = const.tile([128, DIM], F32)

            def load_ln_consts():
                for bc, src_d in ((bout_bc, bout_d), (gamma_bc, gamma_d),
                                  (beta_bc, beta_d)):
                    nc.sync.dma_start(out=bc[:], in_=bass.AP(
                        tensor=src_d.tensor, offset=src_d.offset,
                        ap=[[0, 128], [1, DIM]]))
            # w_out -> bf16 [128, 8, 1024]
            wout_bf = const.tile([128, 8, DIM], BF16)

            # ---------------- persistent activations ----------------
            qT_sb = const.tile([128, ROWS], F32R)   # 4h x 32d on partitions
            kT_sb = const.tile([128, ROWS], F32R)
            # V with a ones column per head: [key%128, ktile, head, 33]
            V_sb = const.tile([128, 2 * KT, HPC, DH + 1], BF16)
            nc.vector.memset(V_sb[:, :, :, DH:DH + 1], 1.0)

            # ---------------- dram bounce buffers ----------------
            a2a_in = [dram.tile([N_CORES, 128, 128], BF16, name=f"a2ai_{k}")
                      for k in range(NCK)]
            a2a_out = [dram.tile([N_CORES, 128, 128], BF16, name=f"a2ao_{k}")
                       for k in range(NCK)]

            # ---------------- phase A: projections ----------------
            def proj_rowchunk(rc, split=False):
                xt = work.tile([128, 8, RC], BF16, tag="xt", bufs=3,
                               name=f"xt_{rc}")
                src = (xT_d[:, rc * RC:(rc + 1) * RC]
                       .rearrange("(kc p) n -> p kc n", p=128))
                if split:
                    # startup: split into 4 DMAs (parallel hw queues) on
                    # sync + 2 on gpsimd
                    for kc in range(3):
                        nc.sync.dma_start(xt[:, 2 * kc:2 * kc + 2, :],
                                          src[:, 2 * kc:2 * kc + 2, :])
                    nc.gpsimd.dma_start(xt[:, 6:8, :], src[:, 6:8, :])
                else:
                    # keep the gpsimd queue free for collectives: even
                    # chunks on sync, odd chunks on the scalar hwdge
                    dma_eng = nc.sync if rc % 2 == 0 else nc.scalar
                    dma_eng.dma_start(xt[:], src)
                for name, mofs, dst in (("q", 0, qT_sb), ("k", 128, kT_sb)):
                    pp = ps.tile([128, RC], F32, tag="sp", bufs=3,
                                 name=f"pp_{name}_{rc}")
                    for kc in range(8):
                        nc.tensor.matmul(
                            pp[:], wqkv_sb[:, kc, mofs:mofs + 128],
                            xt[:, kc, :], start=(kc == 0), stop=(kc == 7))
                    nc.vector.tensor_copy(dst[:, rc * RC:(rc + 1) * RC], pp[:])
                # v: project (vT layout), cast bf16, DMA-transpose into V_sb
                pv_ = ps.tile([128, RC], F32, tag="sp", bufs=3,
                               name=f"pp_v_{rc}")
                for kc in range(8):
                    nc.tensor.matmul(
                        pv_[:], wqkv_sb[:, kc, 256:384], xt[:, kc, :],
                        start=(kc == 0), stop=(kc == 7))
                vt = work.tile([128, RC], BF16, tag="vt", bufs=2,
                               name=f"vt_{rc}")
                nc.vector.tensor_copy(vt[:], pv_[:])
                for i in range(RC // 128):
                    tp = ps.tile([128, 128], BF16, tag="sp", bufs=3,
                                 name=f"tp_{rc}_{i}")
                    nc.tensor.matmul(
                        tp[:], vt[:, i * 128:(i + 1) * 128], ident[:],
                        is_transpose=True, start=True, stop=True)
                    kt_abs = rc * (RC // 128) + i
                    nc.vector.tensor_copy(
                        V_sb[:, kt_abs, :, 0:DH],
                        tp[:].rearrange("p (h d) -> p h d", h=HPC))

            def load_wout(j):
                nc.sync.dma_start(wout_bf[:, j, :],
                                  wout_d[j * 128:(j + 1) * 128, :])

            proj_rowchunk(0, split=True)    # enough rows to start qb0

            # ---------------- phase B: attention ----------------
            # Software-pipelined per q-block: S two key-tiles ahead; ACT
            # exp paces heads 0-1, DVE Schraudolph paces heads 2-3; PV
            # (with fused denominator column) trails by one tile.
            def emit_S(b, qb, kt):
                q0 = b * N + qb * QB
                k0 = b * N + kt * 128
                tA = ps.tile([128, 2, QB], F32, tag="sp", bufs=3,
                             name=f"sA_{b}_{qb}_{kt}")
                tB = ps.tile([128, 2, QB], F32, tag="sp", bufs=3,
                             name=f"sB_{b}_{qb}_{kt}")
                for h in range(4):
                    t = tA if h < 2 else tB
                    nc.tensor.matmul(
                        t[:, h % 2, :],
                        kT_sb[32 * h:32 * h + 32, k0:k0 + 128],
                        qT_sb[32 * h:32 * h + 32, q0:q0 + QB],
                        start=True, stop=True, tile_position=(32 * h, 0))
                return tA, tB

            pending_epi = [None]

            def flush_epi():
                if pending_epi[0] is not None:
                    fn = pending_epi[0]
                    pending_epi[0] = None
                    fn()

            def attention_qblock(b, qb, extra=None):
                # PV+dn accumulator: slot j holds heads {j, j+? } ->
                # head h: partitions [64*(h%2), +33), bank h//2.
                pvp = ps.tile([128, 2, QB], F32, tag="pv", name=f"pv_{b}_{qb}")
                s_tiles = {0: emit_S(b, qb, 0), 1: emit_S(b, qb, 1)}
                flush_epi()   # previous q-block's tail, behind our first S
                for kt in range(KT):
                    if extra is not None:
                        extra(kt)
                    if kt + 2 < KT:
                        s_tiles[kt + 2] = emit_S(b, qb, kt + 2)
                    tA, tB = s_tiles.pop(kt)
                    eA = work.tile([128, 2, QB], BF16, tag="expt", bufs=8,
                                   name=f"eA_{b}_{qb}_{kt}")
                    eB = work.tile([128, 2, QB], BF16, tag="expt", bufs=8,
                                   name=f"eB_{b}_{qb}_{kt}")
                    nc.scalar.activation(eA[:], tA[:], AF.Exp, scale=SCALE)
                    nc.vector.tensor_scalar(
                        out=eB[:].bitcast(I16), in0=tB[:],
                        scalar1=float(SCH_A), scalar2=float(SCH_B),
                        op0=ALU.mult, op1=ALU.add)
                    for h in range(4):
                        rhs = (eA if h < 2 else eB)[:, h % 2, :]
                        p0 = 64 * (h % 2)
                        nc.tensor.matmul(
                            pvp[p0:p0 + DH + 1, h // 2, :],
                            V_sb[:, b * KT + kt, h, :],
                            rhs, start=(kt == 0), stop=(kt == KT - 1),
                            tile_position=(0, p0))
                # drain PSUM fast: dn rows via ACT (one op per partition
                # row covering both slots), att rows split ACT/DVE
                attf = work.tile([128, 2, QB], BF16, tag="attf", bufs=2,
                                 name=f"attf_{b}_{qb}")
                nc.scalar.copy(dn_st[32:33, :, :], pvp[32:33, :, :])
                nc.scalar.copy(dn_st[96:97, :, :], pvp[96:97, :, :])
                nc.vector.tensor_copy(attf[0:32, 0, :], pvp[0:32, 0, :])
                nc.vector.tensor_copy(attf[64:96, 0, :], pvp[64:96, 0, :])
                nc.scalar.copy(attf[0:32, 1, :], pvp[0:32, 1, :])
                nc.scalar.copy(attf[64:96, 1, :], pvp[64:96, 1, :])

                def epilogue():
                    ck = 2 * b + qb // 2
                    j0 = 4 * (qb % 2)
                    rec = work.tile([128, 2, QB], F32, tag="rec", bufs=2,
                                    name=f"rec_{b}_{qb}")
                    nc.vector.reciprocal_approx_fast(out=rec[:],
                                                     in_=dn_st[:])
                    # broadcast rec row 32 -> partitions 0-31 and row 96 ->
                    # 64-95 per slot: bounce the two rows through DRAM and
                    # read back with partition-stride-0 DMAs (no PE matmul,
                    # no psum, no engine-queue cycles; all off-critical)
                    rec_d = dram.tile([2, 2, QB], F32, tag="recd", bufs=2,
                                      name=f"recd_{b}_{qb}")
                    recb_sb = work.tile([128, 2, QB], F32, tag="recs",
                                        bufs=2, name=f"recs_{b}_{qb}")
                    for g in range(2):
                        nc.sync.dma_start(rec_d[g],
                                          rec[32 + 64 * g:33 + 64 * g, :, :])
                    for j in range(2):
                        for g in range(2):
                            src = rec_d[g, j, :]
                            nc.sync.dma_start(
                                recb_sb[64 * g:64 * g + 32, j, :],
                                bass.AP(tensor=src.tensor, offset=src.offset,
                                        ap=[[0, 32], [1, QB]]))
                    att = work.tile([128, 2, QB], BF16, tag="att", bufs=2,
                                    name=f"att_{b}_{qb}")
                    for j in range(2):
                        for g in range(2):
                            sl = slice(64 * g, 64 * g + 32)
                            nc.gpsimd.tensor_tensor(
                                att[sl, j, :], attf[sl, j, :],
                                recb_sb[sl, j, :], ALU.mult)
                    # gather DMA: partitions {0-31, 64-95} -> block rows
                    # [64j+32g, +32) of each of the 4 dram blocks
                    for j in range(2):
                        for g in range(2):
                            r0 = 64 * j + 32 * g
                            nc.sync.dma_start(
                                a2a_in[ck][j0:j0 + 4, r0:r0 + 32, :]
                                .rearrange("j p q -> p j q"),
                                att[64 * g:64 * g + 32, j, :]
                                .rearrange("p (j q) -> p j q", j=4))

                pending_epi[0] = epilogue

            def a2a_exchange(ck):
                nc.gpsimd.collective_compute(
                    "AllToAll", ALU.bypass,
                    replica_groups=[list(range(N_CORES))],
                    ins=[a2a_in[ck].opt()], outs=[a2a_out[ck].opt()])

            # ---------------- phase C: out-proj + LN (per 128 rows) -------
            def outproj_load(ck):
                ab = work.tile([128, 8, 128], BF16, tag="a2asb", bufs=2,
                               name=f"ab_{ck}")
                nc.sync.dma_start(ab[:],
                                  a2a_out[ck].rearrange("i p q -> p i q"))
                osb = work.tile([128, DIM], F32, tag="osb", bufs=2,
                                name=f"osb_{ck}")
                return ab, osb

            def outproj_mm(ck, ab, osb, nb):
                op = ps.tile([128, 512], F32, tag="sp", bufs=3,
                             name=f"op_{ck}_{nb}")
                for i in range(N_CORES):
                    nc.tensor.matmul(
                        op[:], ab[:, i, :],
                        wout_bf[:, i, nb * 512:(nb + 1) * 512],
                        start=(i == 0), stop=(i == N_CORES - 1))
                nc.vector.tensor_tensor(
                    osb[:, nb * 512:(nb + 1) * 512], op[:],
                    bout_bc[:, nb * 512:(nb + 1) * 512], ALU.add)

            def outproj_chunk(ck, tail=False):
                ab, osb = outproj_load(ck)
                for nb in range(2):
                    outproj_mm(ck, ab, osb, nb)
                outproj_ln(ck, osb, tail=tail)

            def outproj_ln(ck, osb, tail=False):
                # LayerNorm over the 1024 free dim
                stats = work.tile([128, 2, 6], F32, tag="stats", bufs=2,
                                  name=f"stats_{ck}")
                for sg in range(2):
                    nc.vector.bn_stats(out=stats[:, sg, :],
                                       in_=osb[:, sg * 512:(sg + 1) * 512])
                mv = work.tile([128, 2], F32, tag="mv", bufs=2,
                               name=f"mv_{ck}")
                nc.vector.bn_aggr(out=mv[:], in_=stats[:])
                # rstd = 1/sqrt(var + eps), DVE-only: float quake seed
                # (bits(y) = C - bits(x)/2 done in fp32) + 2 Newton steps.
                # Keeps the Scalar engine free of Ln (no ACT table swaps).
                lw = work.tile([128, 4], F32, tag="lnw", bufs=2,
                               name=f"lnw_{ck}")
                veps = lw[:, 0:1]
                nc.vector.tensor_scalar(out=veps, in0=mv[:, 1:2],
                                        scalar1=EPS, scalar2=None,
                                        op0=ALU.add)
                nc.vector.tensor_copy(lw[:, 1:2], veps.bitcast(I32))
                rstd = work.tile([128, 1], F32, tag="rstd", bufs=2,
                                 name=f"rstd_{ck}")
                nc.vector.tensor_scalar(
                    out=rstd[:].bitcast(I32), in0=lw[:, 1:2],
                    scalar1=-0.5, scalar2=1597463007.0,
                    op0=ALU.mult, op1=ALU.add)
                for _ in range(2):
                    nc.vector.tensor_tensor(lw[:, 2:3], rstd[:], rstd[:],
                                            ALU.mult)
                    nc.vector.tensor_tensor(lw[:, 2:3], lw[:, 2:3], veps,
                                            ALU.mult)
                    nc.vector.tensor_scalar(
                        out=lw[:, 2:3], in0=lw[:, 2:3], scalar1=-0.5,
                        scalar2=1.5, op0=ALU.mult, op1=ALU.add)
                    nc.vector.tensor_tensor(rstd[:], rstd[:], lw[:, 2:3],
                                            ALU.mult)
                nc.vector.tensor_scalar(
                    out=osb[:], in0=osb[:], scalar1=mv[:, 0:1],
                    scalar2=rstd[:], op0=ALU.subtract, op1=ALU.mult)
                eng = nc.vector if tail else nc.gpsimd
                eng.tensor_tensor(osb[:], osb[:], gamma_bc[:], ALU.mult)
                eng.tensor_tensor(osb[:], osb[:], beta_bc[:], ALU.add)
                nc.sync.dma_start(out_d[ck * 128:(ck + 1) * 128, :], osb[:])

            # ---------------- schedule ----------------
            # per-kt emission hooks: stream the remaining projections and
            # weight loads into the attention pipeline instead of bursts
            def extra_00(kt):     # proj chunks 1..3 (rest of batch 0)
                if kt in (0, 4, 8):
                    proj_rowchunk(1 + kt // 4)

            attention_qblock(0, 0, extra_00)
            load_ln_consts()
            proj_rowchunk(4)
            attention_qblock(0, 1)
            flush_epi()
            a2a_exchange(0)
            proj_rowchunk(5)
            load_wout(0)
            load_wout(1)
            attention_qblock(0, 2)
            proj_rowchunk(6)
            load_wout(2)
            load_wout(3)
            attention_qblock(0, 3)
            flush_epi()
            a2a_exchange(1)
            proj_rowchunk(7)
            load_wout(4)
            load_wout(5)
            attention_qblock(1, 0)
            load_wout(6)
            load_wout(7)
            attention_qblock(1, 1)
            flush_epi()
            a2a_exchange(2)

            op_state = {}

            def extra_op(ck, k0=2):
                def hook(kt):
                    if kt == k0:
                        op_state[ck] = outproj_load(ck)
                    elif kt == k0 + 4:
                        outproj_mm(ck, *op_state[ck], 0)
                    elif kt == k0 + 8:
                        outproj_mm(ck, *op_state[ck], 1)
                    elif kt == k0 + 12:
                        outproj_ln(ck, op_state[ck][1])
                return hook

            def extra_op2(h1, h2):
                def hook(kt):
                    h1(kt)
                    h2(kt)
                return hook

            attention_qblock(1, 2, extra_op(0))
            attention_qblock(1, 3, extra_op2(extra_op(1, 1), extra_op(2, 3)))
            flush_epi()
            a2a_exchange(3)
            outproj_chunk(3, tail=True)

    nc.compile()
    return nc


class _Runner:
    """Compile once; run the SPMD kernel on 8 cores via PJRT repeatedly."""

    def __init__(self):
        self.nc = _build()
        import jax
        from jax.sharding import Mesh, PartitionSpec, NamedSharding
        from jax.experimental.shard_map import shard_map
        from concourse import bass2jax
        bass2jax.install_neuronx_cc_hook()

        nc = self.nc
        part_name = (nc.partition_id_tensor.name
                     if nc.partition_id_tensor else None)
        in_names, out_names, out_avals = [], [], []
        for alloc in nc.m.functions[0].allocations:
            if not isinstance(alloc, mybir.MemoryLocationSet):
                continue
            name = alloc.memorylocations[0].name
            if alloc.kind == "ExternalInput":
                if name != part_name:
                    in_names.append(name)
            elif alloc.kind == "ExternalOutput":
                out_names.append(name)
                out_avals.append(jax.core.ShapedArray(
                    tuple(alloc.tensor_shape), mybir.dt.np(alloc.dtype)))
        self.in_names = list(in_names)
        self.out_names = out_names
        self.out_avals = out_avals
        all_in_names = in_names + out_names
        if part_name is not None:
            all_in_names = all_in_names + [part_name]

        def _body(*args):
            operands = list(args)
            if part_name is not None:
                operands.append(bass2jax.partition_id_tensor())
            outs = bass2jax._bass_exec_p.bind(
                *operands, out_avals=tuple(out_avals),
                in_names=tuple(all_in_names), out_names=tuple(out_names),
                lowering_input_output_aliases=(),
                sim_require_finite=True, sim_require_nnan=True, nc=nc)
            return tuple(outs)

        devices = jax.devices()[:N_CORES]
        mesh = Mesh(np.asarray(devices), ("core",))
        self.sharding = NamedSharding(mesh, PartitionSpec("core"))
        nin = len(self.in_names) + len(out_names)
        self.fn = jax.jit(shard_map(
            _body, mesh=mesh, in_specs=(PartitionSpec("core"),) * nin,
            out_specs=(PartitionSpec("core"),) * len(out_names),
            check_rep=False))
        self.jax = jax

    def stage(self, in_maps):
        """Concatenate per-core inputs + zero outputs; device_put with the
        mesh sharding so steady-state calls skip any resharding."""
        concat = [np.concatenate([m[name] for m in in_maps], axis=0)
                  for name in self.in_names]
        zeros = [np.zeros((N_CORES * a.shape[0], *a.shape[1:]), a.dtype)
                 for a in self.out_avals]
        return [self.jax.device_put(x, self.sharding) for x in concat + zeros]

    def run_staged(self, staged):
        outs = self.fn(*staged)
        self.jax.block_until_ready(outs)
        return outs

    def run(self, in_maps):
        outs = self.run_staged(self.stage(in_maps))
        return [
            {name: np.asarray(outs[i]).reshape(
                N_CORES, *self.out_avals[i].shape)[c]
             for i, name in enumerate(self.out_names)}
            for c in range(N_CORES)
        ]


_RUNNER = None


def _get_runner():
    global _RUNNER
    if _RUNNER is None:
        _RUNNER = _Runner()
    return _RUNNER


def _make_in_maps(x, w_qkv, w_out, b_out, ln_gamma, ln_beta):
    bf16 = mybir.dt.np(BF16)
    x = np.asarray(x, dtype=np.float32)
    w_qkv = np.asarray(w_qkv, dtype=np.float32)
    w_out = np.asarray(w_out, dtype=np.float32).astype(bf16)
    b_out = np.asarray(b_out, dtype=np.float32)
    ln_gamma = np.asarray(ln_gamma, dtype=np.float32)
    ln_beta = np.asarray(ln_beta, dtype=np.float32)

    xT = np.ascontiguousarray(x.reshape(ROWS, DIM).T).astype(bf16)
    in_maps = []
    for c in range(N_CORES):
        h0 = HPC * c * DH
        cols = np.concatenate([
            w_qkv[:, h0:h0 + HPC * DH],
            w_qkv[:, DIM + h0:DIM + h0 + HPC * DH],
            w_qkv[:, 2 * DIM + h0:2 * DIM + h0 + HPC * DH],
        ], axis=1)
        in_maps.append({
            "xT": xT,
            "wqkv": np.ascontiguousarray(cols).astype(bf16),
            "wout": w_out,
            "bout": b_out,
            "gamma": ln_gamma,
            "beta": ln_beta,
        })
    return in_maps


def kernel(x, w_qkv, w_out, b_out, ln_gamma, ln_beta):
    runner = _get_runner()
    in_maps = _make_in_maps(x, w_qkv, w_out, b_out, ln_gamma, ln_beta)
    results = runner.run(in_maps)
    # per-core out rows: [chunk(4), 128]; global row = 1024*ck + 128*c + r
    full = np.empty((ROWS, DIM), dtype=np.float32)
    for c in range(N_CORES):
        o = results[c]["out"]
        for ck in range(NCK):
            r0 = 1024 * ck + 128 * c
            full[r0:r0 + 128] = o[ck * 128:(ck + 1) * 128]
    return full.reshape(B, N, DIM)
